# revision 22
# baseline (speedup 1.0000x reference)
"""Trainium2 Bass kernel for nn_BaselineModel_80796924772520 (dense_cnn).

Self-contained: kernel(**inputs) -> np.ndarray [512, 7] float32.

Strategy: pure data parallelism over 8 NeuronCores (64 images each).
 - BN folded into conv weights/biases on host; fc1/fc2/att collapse into
   one linear map W_eff [64, 2304] on host.
 - conv1 (C_in=1, K=9): 4-way PE row tiling (tile_position=(32i,0), one
   9-row contraction per 32-row tile) -> 4 concurrent matmuls. im2col
   rows live at partitions 32i+k; no contraction zero-padding needed.
 - conv2: 9-tap border-clipped accumulating matmuls over UNPADDED bf16
   activations (center tap first covers the full region with start=True;
   shifted taps accumulate partial rectangles = zero-padding semantics).
 - conv3: same clipping + 2-way PE column tiling (two image pairs run
   concurrently at tile_position=(0,0)/(0,64)).
 - evictions: scalar ACT does relu+bias straight from PSUM into bf16
   SBUF (the only cheap PSUM reader), then 2x2 maxpool = two small DVE
   tensor_max ops at 2x mode (relu/bias commute with max).
 - attention: per-image [64x36]^T@[64x1] matmuls -> PE transpose ->
   softmax -> broadcast-matmul with ones -> multiply+segmented reduce.
"""
import sys
if '/opt/trn_rl_repo' not in sys.path:
    sys.path.insert(0, '/opt/trn_rl_repo')

import contextlib
import numpy as np

import concourse.bass as bass
import concourse.mybir as mybir
import concourse.tile as tile

F32 = mybir.dt.float32
BF16 = mybir.dt.bfloat16
DT_MM = BF16
RELU = mybir.ActivationFunctionType.Relu
EXP = mybir.ActivationFunctionType.Exp

N_CORES = 8
B_TOTAL = 512
BPC = B_TOTAL // N_CORES   # 64 images per core
G = 8                      # images per group
NG = BPC // G              # 8 groups
EPS = 1e-5

# bisection switches (all True = full optimization)
C1_TILED = True    # conv1 4-way PE row tiling
C3_TILED = False   # conv3 2-way PE column tiling
USE_MEMSET = False  # engine memsets for act border zeroing (else DMA bcast)

_MAX_WAITS = 1  # this walrus build supports 1 sync-wait per instruction


def _install_tile_fixups():
    """The nix walrus here allows only ONE sync-wait per instruction; Tile's
    exit drain aggregates one wait per live proc onto a single Drain. Spread
    the waits across spare SP nops emitted just before the drain."""
    if getattr(tile.TileContext, '_drain_patched', False):
        return

    def _patched(self, tick_clock, wait_clock):
        from concourse.vector_clock import ScopedClock
        nc = self.nc
        nops = [nc.sync.nop().ins for _ in range(32)]
        drain_inst = nc.sync.drain()
        wait_clock.add_sem_waits(
            drain_inst.ins, ScopedClock({None: tick_clock.global_clock}))
        si = drain_inst.ins.sync_info
        if si is not None and len(si.on_wait) > _MAX_WAITS:
            waits = list(si.on_wait)
            drain_inst.ins.sync_info = mybir.SyncInfo(
                on_wait=waits[:_MAX_WAITS], on_update=list(si.on_update))
            rest = waits[_MAX_WAITS:]
            for i in range(0, len(rest), _MAX_WAITS):
                nops[i // _MAX_WAITS].sync_info = mybir.SyncInfo(
                    on_wait=rest[i:i + _MAX_WAITS], on_update=[])
        nc.all_engine_barrier()
        popped = nc._tile_sem_poison_stack.pop()
        assert popped is self._sem_poison
        nc.clear_and_free_semaphores(list(self.sems.allocated().values()))
        nc.all_engine_barrier()

    tile.TileContext._drain_and_barrier = _patched
    tile.TileContext._drain_patched = True


def _split_excess_waits(nc):
    """This walrus allows one sync-wait per instruction. Hoist excess waits
    onto same-engine nops inserted immediately before the instruction
    (sequential waits on one engine are equivalent to a combined wait)."""
    idx = 0
    for f in nc.m.functions:
        for b in f.blocks:
            out, changed = [], False
            for ins in b.instructions:
                si = ins.sync_info
                if si is not None and len(si.on_wait) > _MAX_WAITS:
                    waits = list(si.on_wait)
                    extra, keep = waits[:-_MAX_WAITS], waits[-_MAX_WAITS:]
                    for j in range(0, len(extra), _MAX_WAITS):
                        nop = mybir.InstNoOp(name=f"I-wsplit-{idx}")
                        idx += 1
                        nop.engine = ins.engine
                        nop.sync_info = mybir.SyncInfo(
                            on_wait=extra[j:j + _MAX_WAITS], on_update=[])
                        nc.register_instruction(nop, overwrite=True)
                        out.append(nop)
                    ins.sync_info = mybir.SyncInfo(
                        on_wait=keep, on_update=list(si.on_update))
                    changed = True
                out.append(ins)
            if changed:
                b.instructions = out


def _prep_weights(p):
    """Fold BN, collapse FC chain, lay out weights for the device program."""
    def fold(w, b, g, be, m, v):
        inv = (g / np.sqrt(v + EPS)).astype(np.float32)
        wf = (w * inv[:, None, None, None]).astype(np.float32)
        bf = ((b - m) * inv + be).astype(np.float32)
        return wf, bf

    w1, b1 = fold(p['conv1_w'], p['conv1_b'], p['bn1_g'], p['bn1_b'], p['bn1_m'], p['bn1_v'])
    w2, b2 = fold(p['conv2_w'], p['conv2_b'], p['bn2_g'], p['bn2_b'], p['bn2_m'], p['bn2_v'])
    w3, b3 = fold(p['conv3_w'], p['conv3_b'], p['bn3_g'], p['bn3_b'], p['bn3_m'], p['bn3_v'])

    # conv1 lhsT [128, 256]: rows 32i+k (k = 3*ky+kx) = w1[c, 0, ky, kx],
    # replicated into all 4 row-quadrants for 4-way PE row tiling.
    W1T = np.zeros((128, 256), np.float32)
    for i in range(4):
        W1T[32 * i:32 * i + 9, :] = w1.reshape(256, 9).T
    # conv2 lhsT [128, 2304]: [p, t*256 + h*128 + m] = w2[m, 128h+p, t]
    W2T = np.ascontiguousarray(
        w2.reshape(128, 2, 128, 9).transpose(2, 3, 1, 0)  # [p, t, h, m]
    ).reshape(128, 2304)
    # conv3 lhsT [128, 576]: [p, t*64 + m] = w3[m, p, t]
    W3T = np.ascontiguousarray(
        w3.reshape(64, 128, 9).transpose(1, 2, 0)).reshape(128, 576)

    # FC chain collapse: q = out4 @ W_eff.T + b_eff
    fc1w, fc2w, attw = p['fc1_w'], p['fc2_w'], p['att_w']
    W_eff = (attw @ fc2w @ fc1w).astype(np.float32)          # [64, 2304]
    b_eff = (attw @ (fc2w @ p['fc1_b'] + p['fc2_b']) + p['att_b']).astype(np.float32)
    # WeT2 [64, 2304]: [c, hw*64 + m] = W_eff[m, c*36 + hw]
    WeT2 = np.ascontiguousarray(
        W_eff.reshape(64, 64, 36).transpose(1, 2, 0)).reshape(64, 2304)

    W3fT = np.ascontiguousarray(p['fc3_w'].T).astype(np.float32)  # [64, 7]
    fc3b_rep = np.broadcast_to(p['fc3_b'], (64, 7)).astype(np.float32).copy()

    b1c = np.ascontiguousarray(b1.reshape(2, 128).T)       # [128, 2]
    b2c = b2.reshape(128, 1).astype(np.float32)
    b3c = np.concatenate([b3, b3]).reshape(128, 1).astype(np.float32)
    beffc = b_eff.reshape(64, 1).astype(np.float32)

    return dict(W1T=W1T, W2T=W2T, W3T=W3T, WeT2=WeT2, W3fT=W3fT,
                fc3b_rep=fc3b_rep, b1c=b1c, b2c=b2c, b3c=b3c, beffc=beffc,
                Z=np.zeros((1, 12544), np.float32),
                IDENT=np.eye(36, dtype=np.float32),
                ONES=np.ones((1, 64), np.float32))


def build_program(debug=False):
    """Build the per-core SPMD Bass program. Returns nc."""
    _install_tile_fixups()
    nc = bass.Bass("TRN2", target_bir_lowering=False, debug=False)

    x = nc.declare_dram_parameter("x", [BPC, 2304], DT_MM, isOutput=False)
    W1T = nc.declare_dram_parameter("W1T", [128, 256], DT_MM, isOutput=False)
    W2T = nc.declare_dram_parameter("W2T", [128, 2304], DT_MM, isOutput=False)
    W3T = nc.declare_dram_parameter("W3T", [128, 576], DT_MM, isOutput=False)
    WeT2 = nc.declare_dram_parameter("WeT2", [64, 2304], DT_MM, isOutput=False)
    W3fT = nc.declare_dram_parameter("W3fT", [64, 7], DT_MM, isOutput=False)
    fc3b = nc.declare_dram_parameter("fc3b_rep", [64, 7], F32, isOutput=False)
    b1c = nc.declare_dram_parameter("b1c", [128, 2], F32, isOutput=False)
    b2c = nc.declare_dram_parameter("b2c", [128, 1], F32, isOutput=False)
    b3c = nc.declare_dram_parameter("b3c", [128, 1], F32, isOutput=False)
    beffc = nc.declare_dram_parameter("beffc", [64, 1], F32, isOutput=False)
    Z = nc.declare_dram_parameter("Z", [1, 12544], DT_MM, isOutput=False)
    IDENT = nc.declare_dram_parameter("IDENT", [36, 36], F32, isOutput=False)
    ONES = nc.declare_dram_parameter("ONES", [1, 64], DT_MM, isOutput=False)
    out = nc.declare_dram_parameter("out", [BPC, 7], F32, isOutput=True)
    dbg = {}
    if debug:
        for nm, shp in [("dbg_act1_p0h0", [128, G * 676]), ("dbg_act1_p0h1", [128, G * 676]),
                        ("dbg_act1_p1h0", [128, G * 676]), ("dbg_act1_p1h1", [128, G * 676]),
                        ("dbg_act2", [128, BPC * 196]), ("dbg_out3", [64, BPC * 36]),
                        ("dbg_q", [64, 64]), ("dbg_attn", [64, 36]),
                        ("dbg_gT", [64, 64]), ("dbg_sc", [36, 64])]:
            dbg[nm] = nc.declare_dram_parameter(nm, shp, F32, isOutput=True)

    with tile.TileContext(nc) as tc, contextlib.ExitStack() as ctx:
        wp = ctx.enter_context(tc.tile_pool(name="weights", bufs=1))
        ap_pool = ctx.enter_context(tc.tile_pool(name="acts", bufs=1))
        cp = ctx.enter_context(tc.tile_pool(name="im2col", bufs=2))
        e1p = ctx.enter_context(tc.tile_pool(name="ev1", bufs=4))
        e2p = ctx.enter_context(tc.tile_pool(name="ev2", bufs=4))
        e3p = ctx.enter_context(tc.tile_pool(name="ev3", bufs=4))

        # ---- prologue: xpad zero-fill + group-0 x + taps ahead of the
        # bulk weight DMAs ----
        xpads = [ap_pool.tile([8, 2500], DT_MM, tag=f"xpad{pp}",
                              name=f"xpad{pp}") for pp in range(2)]
        for pp in range(2):
            nc.gpsimd.dma_start(out=xpads[pp][:],
                                in_=Z[:, :2500].to_broadcast((8, 2500)))
        xpv0 = xpads[0][:].rearrange("p (y x) -> p y x", x=50)
        nc.sync.dma_start(
            out=xpv0[0:G, 1:49, 1:49],
            in_=x[0:G, :].rearrange("b (y x) -> b y x", x=48))

        w1t = wp.tile([128, 256], DT_MM)
        nc.sync.dma_start(out=w1t[:], in_=W1T[:])
        b1t = wp.tile([128, 2], F32)
        nc.sync.dma_start(out=b1t[:], in_=b1c[:])

        def issue_taps(imt, xpv, g):
            """36 DMAs: tap k of position-block (tile) i -> imt[32i+k].
            Tile i owns output rows 12i..12i+12 (a contiguous quarter);
            imt cols per tap-row = (img, y(12), x(48)). Input = a clean 3D
            slice of xpad, so the DMA balances within 3 dims."""
            n = 0
            for k in range(9):
                dy, dx = divmod(k, 3)
                for i in range(4):
                    eng = (nc.sync, nc.gpsimd)[n % 2]
                    n += 1
                    src = xpv[0:G, :].rearrange("p (y x) -> p y x", x=50)
                    eng.dma_start(
                        out=imt[32 * i + k:32 * i + k + 1, :],
                        in_=src[:, 12 * i + dy:12 * i + dy + 12, dx:dx + 48])
            return imt

        # group-0 im2col ahead of heavy weight loads
        imt0 = cp.tile([128, G * 576], DT_MM, tag="imt", name="imt0")
        issue_taps(imt0, xpads[0][:], 0)

        # ---- remaining weights ----
        w2t = wp.tile([128, 2304], DT_MM)
        nc.sync.dma_start(out=w2t[:], in_=W2T[:])
        w3t = wp.tile([128, 576], DT_MM)
        nc.sync.dma_start(out=w3t[:], in_=W3T[:])
        wet = wp.tile([64, 2304], DT_MM)
        nc.sync.dma_start(out=wet[:], in_=WeT2[:])
        w3f = wp.tile([64, 7], DT_MM)
        nc.sync.dma_start(out=w3f[:], in_=W3fT[:])
        fc3b_t = wp.tile([64, 7], F32)
        nc.sync.dma_start(out=fc3b_t[:], in_=fc3b[:])
        b2t = wp.tile([128, 1], F32)
        nc.sync.dma_start(out=b2t[:], in_=b2c[:])
        b3t = wp.tile([128, 1], F32)
        nc.sync.dma_start(out=b3t[:], in_=b3c[:])
        bet = wp.tile([64, 1], F32)
        nc.sync.dma_start(out=bet[:], in_=beffc[:])
        ident = wp.tile([36, 36], F32)
        nc.sync.dma_start(out=ident[:], in_=IDENT[:])
        ones1 = wp.tile([1, 64], DT_MM)
        nc.sync.dma_start(out=ones1[:], in_=ONES[:])

        # ---- persistent activation buffers (zero-padded interiors; the
        # borders are zeroed ONCE here via engine memsets on otherwise-idle
        # engines — interior writes never touch them) ----
        act1 = [[ap_pool.tile([128, G * 676], DT_MM, tag=f"act1_{pp}_{h}",
                              name=f"act1_{pp}_{h}") for h in range(2)]
                for pp in range(2)]
        act2 = ap_pool.tile([128, BPC * 196], DT_MM)
        out3 = ap_pool.tile([64, BPC * 36], DT_MM)
        if USE_MEMSET:
            for pp in range(2):
                for h in range(2):
                    nc.vector.memset(act1[pp][h][:], 0.0)
            nc.gpsimd.memset(act2[:], 0.0)
        else:
            for pp in range(2):
                for h in range(2):
                    nc.sync.dma_start(
                        out=act1[pp][h][:],
                        in_=Z[:, :G * 676].to_broadcast((128, G * 676)))
            nc.gpsimd.dma_start(out=act2[:],
                                in_=Z[:, :BPC * 196].to_broadcast((128, BPC * 196)))

        with contextlib.ExitStack() as cctx:
            # conv1 psum: one 4-bank tile; the 4 concurrently-tiled matmuls
            # MUST land in 4 distinct banks (same-bank concurrent groups
            # crash the runtime — probed). Single-buffered; eviction latency
            # is hidden by interleaving conv2 sub-bursts between rounds.
            ps1 = cctx.enter_context(tc.tile_pool(name="ps1", bufs=1, space="PSUM"))
            ps2 = cctx.enter_context(tc.tile_pool(name="ps2", bufs=2, space="PSUM"))

            def conv1_round(g, ci, imt, h, r):
                """One conv1 round: tile i computes output rows 12i+4r..+4 of
                half h, then relu-first eviction + 2x2 pool into act1."""
                imtv = imt[:].rearrange("p (b y x) -> p b y x", b=G, x=48)
                a1v = act1[g % 2][h][:].rearrange(
                    "p (b y x) -> p b y x", y=26, x=26)
                ps = ps1.tile([128, 2048], F32, tag="ps1", name="ps1")
                for i in range(4):
                    nc.tensor.matmul(
                        out=ps[:, 512 * i:512 * i + 192],
                        lhsT=w1t[32 * i:32 * i + 9, 128 * h:128 * (h + 1)],
                        rhs=imtv[32 * i:32 * i + 9, ci, 4 * r:4 * r + 4, :],
                        start=True, stop=True,
                        tile_position=(32 * i, 0))
                # relu+bias from PSUM -> bf16 (4 tiles x 4 rows x 48)
                blk = e1p.tile([128, 768], DT_MM, tag="blk", name="blk")
                nc.scalar.activation(
                    out=blk[:],
                    in_=ps[:].rearrange("p (i c) -> p i c", c=512)[:, :, 0:192],
                    func=RELU, bias=b1t[:, h:h + 1])
                bv = blk[:].rearrange("p (i y x t) -> p i y x t",
                                      i=4, x=24, t=2)
                sbx = e1p.tile([128, 384], DT_MM, tag="sbx", name="sbx")
                sxv = sbx[:].rearrange("p (i y x) -> p i y x", i=4, x=24)
                nc.vector.tensor_max(sxv, bv[:, :, :, :, 0], bv[:, :, :, :, 1])
                sxp = sbx[:].rearrange("p (i y t x) -> p i y t x",
                                       i=4, y=2, t=2, x=24)
                # tile i's pooled rows land at act1 interior rows
                # 1 + 6i + 2r .. +2 (cols 1:25)
                dst = (a1v[:, ci, 1:25, 1:25]
                       .rearrange("p (i y) x -> p i y x", i=4)
                       [:, :, 2 * r:2 * r + 2, :])
                nc.vector.tensor_max(
                    dst, sxp[:, :, :, 0, :], sxp[:, :, :, 1, :])

            def conv2_bursts(g, bb):
                """conv2 for image bb of group g as 6 sub-burst closures of
                6 matmuls each (2 psum groups of 18); the last also evicts."""
                a1vs = [act1[g % 2][h][:].rearrange(
                    "p (b y x) -> p b y x", y=26, x=26) for h in range(2)]
                a2v = act2[:].rearrange("p (b y x) -> p b y x", y=14, x=14)
                ps = ps2.tile([128, 1024], F32, tag="ps2", name="ps2")
                psr = ps[:].rearrange("p (rr c) -> p rr c", c=512)
                mms = []
                for rr in range(2):
                    for t in range(9):
                        dy, dx = divmod(t, 3)
                        for h in range(2):
                            mms.append((rr, t, h, dy, dx))

                def emit_mm(idx):
                    rr, t, h, dy, dx = mms[idx]
                    n = idx % 18
                    nc.tensor.matmul(
                        out=psr[:, rr, 0:288],
                        lhsT=w2t[:, (t * 2 + h) * 128:(t * 2 + h + 1) * 128],
                        rhs=a1vs[h][:, bb, 12 * rr + dy:12 * rr + dy + 12,
                                    dx:dx + 24],
                        start=(n == 0), stop=(n == 17))

                def evict():
                    sb2 = e2p.tile([128, 576], DT_MM, tag="sb2", name="sb2")
                    nc.scalar.activation(
                        out=sb2[:], in_=psr[:, :, 0:288],
                        func=RELU, bias=b2t[:])
                    s2v = sb2[:].rearrange("p (y x t) -> p y x t", x=12, t=2)
                    xm = e2p.tile([128, 288], DT_MM, tag="xm2", name="xm2")
                    xmv = xm[:].rearrange("p (y x) -> p y x", x=12)
                    nc.vector.tensor_max(xmv, s2v[:, :, :, 0], s2v[:, :, :, 1])
                    xmp = xm[:].rearrange("p (y t x) -> p y t x", t=2, x=12)
                    nc.vector.tensor_max(
                        a2v[:, G * g + bb, 1:13, 1:13],
                        xmp[:, :, 0, :], xmp[:, :, 1, :])

                def burst(j):
                    def run():
                        for idx in range(6 * j, 6 * j + 6):
                            emit_mm(idx)
                        if j == 5:
                            evict()
                    return run
                return [burst(j) for j in range(6)]

            # ---- group loop: conv1(g) rounds interleaved with conv2(g-1)
            # sub-bursts (the conv2 matmuls cover conv1's psum eviction
            # latency between single-buffered rounds) ----
            prev_g = None
            for g in range(NG):
                if g > 0:
                    xpv = xpads[g % 2][:].rearrange("p (y x) -> p y x", x=50)
                    nc.sync.dma_start(
                        out=xpv[0:G, 1:49, 1:49],
                        in_=x[G * g:G * (g + 1), :].rearrange("b (y x) -> b y x", x=48))
                    imt = cp.tile([128, G * 576], DT_MM, tag="imt", name="imt")
                    issue_taps(imt, xpads[g % 2][:], g)
                else:
                    imt = imt0
                for ci in range(G):
                    bursts = (conv2_bursts(prev_g, ci)
                              if prev_g is not None else [None] * 6)
                    for j, (h, r) in enumerate(
                            [(h, r) for h in range(2) for r in range(3)]):
                        if bursts[j] is not None:
                            bursts[j]()
                        conv1_round(g, ci, imt, h, r)
                prev_g = g
            for ci in range(G):
                for b in conv2_bursts(prev_g, ci):
                    b()

        # ---- conv3 (act2 complete): 2-way column tiling ----
        with contextlib.ExitStack() as cctx:
            ps3 = cctx.enter_context(tc.tile_pool(name="ps3", bufs=3, space="PSUM"))
            psq = cctx.enter_context(tc.tile_pool(name="psq", bufs=1, space="PSUM"))
            pssc = cctx.enter_context(tc.tile_pool(name="pssc", bufs=1, space="PSUM"))
            psT = cctx.enter_context(tc.tile_pool(name="psT", bufs=1, space="PSUM"))

            a2v = act2[:].rearrange("p (b y x) -> p b y x", y=14, x=14)
            o3v = out3[:].rearrange("p (b hw) -> p b hw", hw=36)
            if C3_TILED:
                for t in range(16):  # rounds of 4 images (2 col-tiled pairs)
                    ps = ps3.tile([128, 288], F32, tag="ps3", name="ps3")
                    for j in range(2):
                        for k in range(9):
                            dy, dx = divmod(k, 3)
                            nc.tensor.matmul(
                                out=ps[64 * j:64 * (j + 1), :],
                                lhsT=w3t[:, 64 * k:64 * (k + 1)],
                                rhs=a2v[:, 4 * t + 2 * j:4 * t + 2 * j + 2,
                                        dy:dy + 12, dx:dx + 12],
                                start=(k == 0), stop=(k == 8),
                                tile_position=(0, 64 * j))
                    sb3 = e3p.tile([128, 288], DT_MM, tag="sb3", name="sb3")
                    nc.scalar.activation(out=sb3[:], in_=ps[:], func=RELU,
                                         bias=b3t[:])
                    s3v = sb3[:].rearrange("p (b y x t) -> p b y x t", b=2, x=6, t=2)
                    xm = e3p.tile([128, 144], DT_MM, tag="xm3", name="xm3")
                    xmv = xm[:].rearrange("p (b y x) -> p b y x", b=2, x=6)
                    nc.vector.tensor_max(xmv, s3v[:, :, :, :, 0], s3v[:, :, :, :, 1])
                    xmp = xm[:].rearrange("p (b y t x) -> p b y t x", b=2, t=2, x=6)
                    # top pair -> out3 directly; bottom pair pools into a
                    # staging tile on partitions 64-127, then a tiny SBUF->
                    # SBUF DMA moves it down (DVE lanes are fixed).
                    dst = o3v[:, 4 * t:4 * t + 2, :].rearrange(
                        "p b (y x) -> p b y x", x=6)
                    nc.vector.tensor_max(
                        dst, xmp[0:64, :, :, 0, :], xmp[0:64, :, :, 1, :])
                    stg = e3p.tile([128, 72], DT_MM, tag="stg3", name="stg3")
                    sgv = stg[:].rearrange("p (b y x) -> p b y x", b=2, x=6)
                    nc.vector.tensor_max(
                        sgv[64:128, :, :, :],
                        xmp[64:128, :, :, 0, :], xmp[64:128, :, :, 1, :])
                    nc.gpsimd.dma_start(
                        out=out3[:, 36 * (4 * t + 2):36 * (4 * t + 4)],
                        in_=stg[64:128, :])
            else:
                for t in range(32):  # one image pair per round
                    ps = ps3.tile([64, 288], F32, tag="ps3", name="ps3")
                    for k in range(9):
                        dy, dx = divmod(k, 3)
                        nc.tensor.matmul(
                            out=ps[:],
                            lhsT=w3t[:, 64 * k:64 * (k + 1)],
                            rhs=a2v[:, 2 * t:2 * t + 2, dy:dy + 12, dx:dx + 12],
                            start=(k == 0), stop=(k == 8))
                    sb3 = e3p.tile([64, 288], DT_MM, tag="sb3", name="sb3")
                    nc.scalar.activation(out=sb3[:], in_=ps[:], func=RELU,
                                         bias=b3t[0:64, :])
                    s3v = sb3[:].rearrange("p (b y x t) -> p b y x t", b=2, x=6, t=2)
                    xm = e3p.tile([64, 144], DT_MM, tag="xm3", name="xm3")
                    xmv = xm[:].rearrange("p (b y x) -> p b y x", b=2, x=6)
                    nc.vector.tensor_max(xmv, s3v[:, :, :, :, 0], s3v[:, :, :, :, 1])
                    xmp = xm[:].rearrange("p (b y t x) -> p b y t x", b=2, t=2, x=6)
                    dst = o3v[:, 2 * t:2 * t + 2, :].rearrange(
                        "p b (y x) -> p b y x", x=6)
                    nc.vector.tensor_max(
                        dst, xmp[:, :, :, 0, :], xmp[:, :, :, 1, :])

            # ---- q = W_eff @ out4 + b_eff : accumulate over hw ----
            psq_t = psq.tile([64, 64], F32)
            for hw in range(36):
                nc.tensor.matmul(
                    out=psq_t[:],
                    lhsT=wet[:, 64 * hw:64 * (hw + 1)],
                    rhs=out3[:, hw:2304:36],
                    start=(hw == 0), stop=(hw == 35))
            q_sb = ap_pool.tile([64, 64], DT_MM)
            nc.vector.tensor_scalar_add(q_sb[:], psq_t[:], bet[:])

            # ---- scores: per-image matmuls -> [36, 64] psum ----
            pssc_t = pssc.tile([36, 64], F32)
            for b in range(BPC):
                nc.tensor.matmul(
                    out=pssc_t[:, b:b + 1],
                    lhsT=out3[:, 36 * b:36 * (b + 1)],
                    rhs=q_sb[:, b:b + 1],
                    start=True, stop=True)
            sc_sb = ap_pool.tile([36, 64], F32)
            nc.vector.tensor_copy(sc_sb[:], pssc_t[:])
            psT_t = psT.tile([64, 36], F32)
            nc.tensor.transpose(psT_t[:], sc_sb[:], ident[:])

            # ---- softmax over hw (free dim) ----
            mx = ap_pool.tile([64, 1], F32)
            nc.vector.tensor_reduce(out=mx[:], in_=psT_t[:],
                                    op=mybir.AluOpType.max,
                                    axis=mybir.AxisListType.X)
            nmx = ap_pool.tile([64, 1], F32)
            nc.vector.tensor_scalar_mul(nmx[:], mx[:], -1.0)
            e_t = ap_pool.tile([64, 36], F32)
            nc.scalar.activation(out=e_t[:], in_=psT_t[:], func=EXP, bias=nmx[:])
            z = ap_pool.tile([64, 1], F32)
            nc.vector.tensor_reduce(out=z[:], in_=e_t[:],
                                    op=mybir.AluOpType.add,
                                    axis=mybir.AxisListType.X)
            rz = ap_pool.tile([64, 1], F32)
            nc.vector.reciprocal(rz[:], z[:])
            attn = ap_pool.tile([64, 36], DT_MM)
            nc.vector.tensor_scalar_mul(attn[:], e_t[:], rz[:])
            if debug:
                for pp in range(2):
                    for h in range(2):
                        nc.gpsimd.dma_start(out=dbg[f"dbg_act1_p{pp}h{h}"][:],
                                            in_=act1[pp][h][:])
                nc.gpsimd.dma_start(out=dbg["dbg_act2"][:], in_=act2[:])
                nc.gpsimd.dma_start(out=dbg["dbg_out3"][:], in_=out3[:])
                nc.gpsimd.dma_start(out=dbg["dbg_q"][:], in_=q_sb[:])
                nc.gpsimd.dma_start(out=dbg["dbg_attn"][:], in_=attn[:])
                nc.sync.dma_start(out=dbg["dbg_sc"][:], in_=sc_sb[:])

        # ---- g_mod + fc3 ----
        with contextlib.ExitStack() as cctx:
            psab = cctx.enter_context(tc.tile_pool(name="psab", bufs=1, space="PSUM"))
            psf = cctx.enter_context(tc.tile_pool(name="psf", bufs=1, space="PSUM"))

            attn_flat = ap_pool.tile([1, 2304], DT_MM)
            nc.sync.dma_start(out=attn_flat[:], in_=attn[:])
            psab_t = psab.tile([64, 2304], F32)
            for c in range(5):
                lo = 512 * c
                hi = min(lo + 512, 2304)
                nc.tensor.matmul(out=psab_t[:, lo:hi], lhsT=ones1[:],
                                 rhs=attn_flat[:, lo:hi], start=True, stop=True)
            # in-place: out3 is not needed after this product
            nc.vector.tensor_mul(out3[:], out3[:], psab_t[:])
            gT = ap_pool.tile([64, 64], DT_MM)
            with nc.allow_low_precision(reason="bf16 attn-weighted sum"):
                nc.vector.tensor_reduce(
                    out=gT[:], in_=out3[:].rearrange("p (b hw) -> p b hw", hw=36),
                    op=mybir.AluOpType.add, axis=mybir.AxisListType.X)

            if debug:
                nc.gpsimd.dma_start(out=dbg["dbg_gT"][:], in_=gT[:])
            psf_t = psf.tile([64, 7], F32)
            nc.tensor.matmul(out=psf_t[:], lhsT=gT[:],
                             rhs=w3f[:], start=True, stop=True)
            out_sb = ap_pool.tile([64, 7], F32)
            nc.vector.tensor_add(out_sb[:], psf_t[:], fc3b_t[:])
            nc.sync.dma_start(out=out[:], in_=out_sb[:])

    _split_excess_waits(nc)
    return nc


def kernel(**inputs):
    from concourse.bass_utils import run_bass_kernel_spmd

    w = _prep_weights({k: np.asarray(v, np.float32) for k, v in inputs.items()
                       if k != 'x'})
    npdt = mybir.dt.np(DT_MM)
    for k in ('W1T', 'W2T', 'W3T', 'WeT2', 'W3fT', 'Z', 'ONES'):
        w[k] = w[k].astype(npdt)
    xs = np.asarray(inputs['x'], np.float32).reshape(B_TOTAL, 2304).astype(npdt)

    nc = build_program()
    in_maps = []
    for c in range(N_CORES):
        m = {'x': np.ascontiguousarray(xs[BPC * c:BPC * (c + 1)])}
        m.update({k: v for k, v in w.items()})
        in_maps.append(m)
    res = run_bass_kernel_spmd(nc, in_maps, list(range(N_CORES)))
    outs = [res.results[c]['out'] for c in range(N_CORES)]
    return np.concatenate(outs, axis=0).astype(np.float32)


if __name__ == '__main__':
    rng = np.random.default_rng(0)
    fake = {
        'x': rng.standard_normal((512, 1, 48, 48), dtype=np.float32),
        'conv1_w': rng.standard_normal((256, 1, 3, 3), dtype=np.float32) * 0.05,
        'conv1_b': np.zeros(256, np.float32),
        'bn1_g': np.ones(256, np.float32), 'bn1_b': np.zeros(256, np.float32),
        'bn1_m': np.zeros(256, np.float32), 'bn1_v': np.ones(256, np.float32),
        'conv2_w': rng.standard_normal((128, 256, 3, 3), dtype=np.float32) * 0.05,
        'conv2_b': np.zeros(128, np.float32),
        'bn2_g': np.ones(128, np.float32), 'bn2_b': np.zeros(128, np.float32),
        'bn2_m': np.zeros(128, np.float32), 'bn2_v': np.ones(128, np.float32),
        'conv3_w': rng.standard_normal((64, 128, 3, 3), dtype=np.float32) * 0.05,
        'conv3_b': np.zeros(64, np.float32),
        'bn3_g': np.ones(64, np.float32), 'bn3_b': np.zeros(64, np.float32),
        'bn3_m': np.zeros(64, np.float32), 'bn3_v': np.ones(64, np.float32),
        'fc1_w': rng.standard_normal((512, 2304), dtype=np.float32) * 0.05,
        'fc1_b': np.zeros(512, np.float32),
        'fc2_w': rng.standard_normal((256, 512), dtype=np.float32) * 0.05,
        'fc2_b': np.zeros(256, np.float32),
        'att_w': rng.standard_normal((64, 256), dtype=np.float32) * 0.05,
        'att_b': np.zeros(64, np.float32),
        'fc3_w': rng.standard_normal((7, 64), dtype=np.float32) * 0.05,
        'fc3_b': np.zeros(7, np.float32),
    }
    print(kernel(**fake).shape)


# revision 33
# speedup vs baseline: 1.3120x; 1.3120x over previous
"""Trainium2 Bass kernel for nn_BaselineModel_80796924772520 (dense_cnn).

Self-contained: kernel(**inputs) -> np.ndarray [512, 7] float32.

Strategy: pure data parallelism over 8 NeuronCores (64 images each).
 - BN folded into conv weights/biases on host; fc1/fc2/att collapse into
   one linear map W_eff [64, 2304] on host.
 - conv1 (C_in=1, K=9): 4-way PE row tiling (tile_position=(32i,0), one
   9-row contraction per 32-row tile) -> 4 concurrent matmuls. im2col
   rows live at partitions 32i+k; no contraction zero-padding needed.
 - conv2: 9-tap border-clipped accumulating matmuls over UNPADDED bf16
   activations (center tap first covers the full region with start=True;
   shifted taps accumulate partial rectangles = zero-padding semantics).
 - conv3: same clipping + 2-way PE column tiling (two image pairs run
   concurrently at tile_position=(0,0)/(0,64)).
 - evictions: scalar ACT does relu+bias straight from PSUM into bf16
   SBUF (the only cheap PSUM reader), then 2x2 maxpool = two small DVE
   tensor_max ops at 2x mode (relu/bias commute with max).
 - attention: per-image [64x36]^T@[64x1] matmuls -> PE transpose ->
   softmax -> broadcast-matmul with ones -> multiply+segmented reduce.
"""
import sys
if '/opt/trn_rl_repo' not in sys.path:
    sys.path.insert(0, '/opt/trn_rl_repo')

import contextlib
import numpy as np

import concourse.bass as bass
import concourse.mybir as mybir
import concourse.tile as tile

F32 = mybir.dt.float32
BF16 = mybir.dt.bfloat16
DT_MM = BF16
RELU = mybir.ActivationFunctionType.Relu
EXP = mybir.ActivationFunctionType.Exp

N_CORES = 8
B_TOTAL = 512
BPC = B_TOTAL // N_CORES   # 64 images per core
G = 8                      # images per group
NG = BPC // G              # 8 groups
EPS = 1e-5

# switches
C3_TILED = False   # conv3 2-way PE column tiling
USE_MEMSET = False  # engine memsets for act border zeroing (else DMA bcast)

_MAX_WAITS = 1  # this walrus build supports 1 sync-wait per instruction


def _install_tile_fixups():
    """The nix walrus here allows only ONE sync-wait per instruction; Tile's
    exit drain aggregates one wait per live proc onto a single Drain. Spread
    the waits across spare SP nops emitted just before the drain."""
    if getattr(tile.TileContext, '_drain_patched', False):
        return

    def _patched(self, tick_clock, wait_clock):
        from concourse.vector_clock import ScopedClock
        nc = self.nc
        nops = [nc.sync.nop().ins for _ in range(32)]
        drain_inst = nc.sync.drain()
        wait_clock.add_sem_waits(
            drain_inst.ins, ScopedClock({None: tick_clock.global_clock}))
        si = drain_inst.ins.sync_info
        if si is not None and len(si.on_wait) > _MAX_WAITS:
            waits = list(si.on_wait)
            drain_inst.ins.sync_info = mybir.SyncInfo(
                on_wait=waits[:_MAX_WAITS], on_update=list(si.on_update))
            rest = waits[_MAX_WAITS:]
            for i in range(0, len(rest), _MAX_WAITS):
                nops[i // _MAX_WAITS].sync_info = mybir.SyncInfo(
                    on_wait=rest[i:i + _MAX_WAITS], on_update=[])
        nc.all_engine_barrier()
        popped = nc._tile_sem_poison_stack.pop()
        assert popped is self._sem_poison
        nc.clear_and_free_semaphores(list(self.sems.allocated().values()))
        nc.all_engine_barrier()

    tile.TileContext._drain_and_barrier = _patched
    tile.TileContext._drain_patched = True


def _split_excess_waits(nc):
    """This walrus allows one sync-wait per instruction. Hoist excess waits
    onto same-engine nops inserted immediately before the instruction
    (sequential waits on one engine are equivalent to a combined wait)."""
    idx = 0
    for f in nc.m.functions:
        for b in f.blocks:
            out, changed = [], False
            for ins in b.instructions:
                si = ins.sync_info
                if si is not None and len(si.on_wait) > _MAX_WAITS:
                    waits = list(si.on_wait)
                    extra, keep = waits[:-_MAX_WAITS], waits[-_MAX_WAITS:]
                    for j in range(0, len(extra), _MAX_WAITS):
                        nop = mybir.InstNoOp(name=f"I-wsplit-{idx}")
                        idx += 1
                        nop.engine = ins.engine
                        nop.sync_info = mybir.SyncInfo(
                            on_wait=extra[j:j + _MAX_WAITS], on_update=[])
                        nc.register_instruction(nop, overwrite=True)
                        out.append(nop)
                    ins.sync_info = mybir.SyncInfo(
                        on_wait=keep, on_update=list(si.on_update))
                    changed = True
                out.append(ins)
            if changed:
                b.instructions = out


def _prep_weights(p):
    """Fold BN, collapse FC chain, lay out weights for the device program."""
    def fold(w, b, g, be, m, v):
        inv = (g / np.sqrt(v + EPS)).astype(np.float32)
        wf = (w * inv[:, None, None, None]).astype(np.float32)
        bf = ((b - m) * inv + be).astype(np.float32)
        return wf, bf

    w1, b1 = fold(p['conv1_w'], p['conv1_b'], p['bn1_g'], p['bn1_b'], p['bn1_m'], p['bn1_v'])
    w2, b2 = fold(p['conv2_w'], p['conv2_b'], p['bn2_g'], p['bn2_b'], p['bn2_m'], p['bn2_v'])
    w3, b3 = fold(p['conv3_w'], p['conv3_b'], p['bn3_g'], p['bn3_b'], p['bn3_m'], p['bn3_v'])

    # conv1 lhsT [128, 256]: rows 64+k (k = 3*ky+kx) = w1[c, 0, ky, kx];
    # rows 0-63 stay zero (K padded to 73 keeps the PE in full 128-mode,
    # and the zero region [0:64] is quadrant-aligned for engine memsets
    # while staying disjoint from the tap-DMA rows 64-72).
    W1T = np.zeros((128, 256), np.float32)
    W1T[64:73, :] = w1.reshape(256, 9).T
    # conv2 lhsT [128, 2304]: [p, t*256 + h*128 + m] = w2[m, 128h+p, t]
    W2T = np.ascontiguousarray(
        w2.reshape(128, 2, 128, 9).transpose(2, 3, 1, 0)  # [p, t, h, m]
    ).reshape(128, 2304)
    # conv3 lhsT [128, 576]: [p, t*64 + m] = w3[m, p, t]
    W3T = np.ascontiguousarray(
        w3.reshape(64, 128, 9).transpose(1, 2, 0)).reshape(128, 576)

    # FC chain collapse: q = out4 @ W_eff.T + b_eff
    fc1w, fc2w, attw = p['fc1_w'], p['fc2_w'], p['att_w']
    W_eff = (attw @ fc2w @ fc1w).astype(np.float32)          # [64, 2304]
    b_eff = (attw @ (fc2w @ p['fc1_b'] + p['fc2_b']) + p['att_b']).astype(np.float32)
    # WeT2 [64, 2304]: [c, hw*64 + m] = W_eff[m, c*36 + hw]
    WeT2 = np.ascontiguousarray(
        W_eff.reshape(64, 64, 36).transpose(1, 2, 0)).reshape(64, 2304)

    W3fT = np.ascontiguousarray(p['fc3_w'].T).astype(np.float32)  # [64, 7]
    fc3b_rep = np.broadcast_to(p['fc3_b'], (64, 7)).astype(np.float32).copy()

    b1c = np.ascontiguousarray(b1.reshape(2, 128).T)       # [128, 2]
    b2c = b2.reshape(128, 1).astype(np.float32)
    b3c = np.concatenate([b3, b3]).reshape(128, 1).astype(np.float32)
    beffc = b_eff.reshape(64, 1).astype(np.float32)

    return dict(W1T=W1T, W2T=W2T, W3T=W3T, WeT2=WeT2, W3fT=W3fT,
                fc3b_rep=fc3b_rep, b1c=b1c, b2c=b2c, b3c=b3c, beffc=beffc,
                Z=np.zeros((1, 18432), np.float32),
                IDENT=np.eye(36, dtype=np.float32),
                ONES=np.ones((1, 64), np.float32))


def build_program(debug=False):
    """Build the per-core SPMD Bass program. Returns nc."""
    _install_tile_fixups()
    nc = bass.Bass("TRN2", target_bir_lowering=False, debug=False)

    x = nc.declare_dram_parameter("x", [BPC, 2304], DT_MM, isOutput=False)
    W1T = nc.declare_dram_parameter("W1T", [128, 256], DT_MM, isOutput=False)
    W2T = nc.declare_dram_parameter("W2T", [128, 2304], DT_MM, isOutput=False)
    W3T = nc.declare_dram_parameter("W3T", [128, 576], DT_MM, isOutput=False)
    WeT2 = nc.declare_dram_parameter("WeT2", [64, 2304], DT_MM, isOutput=False)
    W3fT = nc.declare_dram_parameter("W3fT", [64, 7], DT_MM, isOutput=False)
    fc3b = nc.declare_dram_parameter("fc3b_rep", [64, 7], F32, isOutput=False)
    b1c = nc.declare_dram_parameter("b1c", [128, 2], F32, isOutput=False)
    b2c = nc.declare_dram_parameter("b2c", [128, 1], F32, isOutput=False)
    b3c = nc.declare_dram_parameter("b3c", [128, 1], F32, isOutput=False)
    beffc = nc.declare_dram_parameter("beffc", [64, 1], F32, isOutput=False)
    Z = nc.declare_dram_parameter("Z", [1, 18432], DT_MM, isOutput=False)
    IDENT = nc.declare_dram_parameter("IDENT", [36, 36], F32, isOutput=False)
    ONES = nc.declare_dram_parameter("ONES", [1, 64], DT_MM, isOutput=False)
    out = nc.declare_dram_parameter("out", [BPC, 7], F32, isOutput=True)
    dbg = {}
    if debug:
        for nm, shp in [("dbg_act1_p0h0", [128, G * 676]), ("dbg_act1_p0h1", [128, G * 676]),
                        ("dbg_act1_p1h0", [128, G * 676]), ("dbg_act1_p1h1", [128, G * 676]),
                        ("dbg_act2", [128, BPC * 196]), ("dbg_out3", [64, BPC * 36]),
                        ("dbg_q", [64, 64]), ("dbg_attn", [64, 36]),
                        ("dbg_gT", [64, 64]), ("dbg_sc", [36, 64])]:
            dbg[nm] = nc.declare_dram_parameter(nm, shp, F32, isOutput=True)

    with tile.TileContext(nc) as tc, contextlib.ExitStack() as ctx:
        wp = ctx.enter_context(tc.tile_pool(name="weights", bufs=1))
        ap_pool = ctx.enter_context(tc.tile_pool(name="acts", bufs=1))
        cp = ctx.enter_context(tc.tile_pool(name="im2col", bufs=2))
        e1p = ctx.enter_context(tc.tile_pool(name="ev1", bufs=4))
        e2p = ctx.enter_context(tc.tile_pool(name="ev2", bufs=4))
        e3p = ctx.enter_context(tc.tile_pool(name="ev3", bufs=4))

        # ---- prologue: xpad zero-fill + group-0 x + taps ahead of the
        # bulk weight DMAs ----
        xpads = [ap_pool.tile([8, 2500], DT_MM, tag=f"xpad{pp}",
                              name=f"xpad{pp}") for pp in range(2)]
        for pp in range(2):
            nc.gpsimd.dma_start(out=xpads[pp][:],
                                in_=Z[:, :2500].to_broadcast((8, 2500)))
        xpv0 = xpads[0][:].rearrange("p (y x) -> p y x", x=50)
        nc.sync.dma_start(
            out=xpv0[0:G, 1:49, 1:49],
            in_=x[0:G, :].rearrange("b (y x) -> b y x", x=48))

        w1t = wp.tile([128, 256], DT_MM)
        nc.sync.dma_start(out=w1t[:], in_=W1T[:])
        b1t = wp.tile([128, 2], F32)
        nc.sync.dma_start(out=b1t[:], in_=b1c[:])

        # two persistent im2col slots; rows 0-63 zeroed once here (the
        # taps only ever rewrite rows 64-72; K padded to 73 keeps the PE
        # in full 128-mode)
        imts = [ap_pool.tile([73, G * 2304], DT_MM, tag=f"imts{j}",
                             name=f"imts{j}") for j in range(2)]
        for j in range(2):
            for q, eng in enumerate((nc.scalar, nc.gpsimd, nc.sync,
                                     nc.scalar)):
                eng.dma_start(
                    out=imts[j][16 * q:16 * (q + 1), :],
                    in_=Z[:].to_broadcast((16, G * 2304)))

        def issue_taps(imt, xpv, g):
            """9 DMAs: tap k -> imt row 64+k; cols = (img, y(48), x(48))."""
            for k in range(9):
                dy, dx = divmod(k, 3)
                eng = (nc.sync, nc.gpsimd)[k % 2]
                src = xpv[0:G, :].rearrange("p (y x) -> p y x", x=50)
                eng.dma_start(
                    out=imt[64 + k:64 + k + 1, :],
                    in_=src[:, dy:dy + 48, dx:dx + 48])
            return imt

        # group-0 im2col ahead of heavy weight loads
        issue_taps(imts[0], xpads[0][:], 0)

        # ---- remaining weights ----
        w2t = wp.tile([128, 2304], DT_MM)
        nc.sync.dma_start(out=w2t[:], in_=W2T[:])
        w3t = wp.tile([128, 576], DT_MM)
        nc.sync.dma_start(out=w3t[:], in_=W3T[:])
        wet = wp.tile([64, 2304], DT_MM)
        nc.sync.dma_start(out=wet[:], in_=WeT2[:])
        w3f = wp.tile([64, 7], DT_MM)
        nc.sync.dma_start(out=w3f[:], in_=W3fT[:])
        fc3b_t = wp.tile([64, 7], F32)
        nc.sync.dma_start(out=fc3b_t[:], in_=fc3b[:])
        b2t = wp.tile([128, 1], F32)
        nc.sync.dma_start(out=b2t[:], in_=b2c[:])
        b3t = wp.tile([128, 1], F32)
        nc.sync.dma_start(out=b3t[:], in_=b3c[:])
        bet = wp.tile([64, 1], F32)
        nc.sync.dma_start(out=bet[:], in_=beffc[:])
        ident = wp.tile([36, 36], F32)
        nc.sync.dma_start(out=ident[:], in_=IDENT[:])
        ones1 = wp.tile([1, 64], DT_MM)
        nc.sync.dma_start(out=ones1[:], in_=ONES[:])

        # ---- persistent activation buffers (zero-padded interiors; the
        # borders are zeroed ONCE here via engine memsets on otherwise-idle
        # engines — interior writes never touch them) ----
        act1 = [[ap_pool.tile([128, G * 676], DT_MM, tag=f"act1_{pp}_{h}",
                              name=f"act1_{pp}_{h}") for h in range(2)]
                for pp in range(2)]
        act2 = ap_pool.tile([128, BPC * 196], DT_MM)
        out3 = ap_pool.tile([64, BPC * 36], DT_MM)
        if USE_MEMSET:
            for pp in range(2):
                for h in range(2):
                    nc.vector.memset(act1[pp][h][:], 0.0)
            nc.gpsimd.memset(act2[:], 0.0)
        else:
            for pp in range(2):
                for h in range(2):
                    nc.sync.dma_start(
                        out=act1[pp][h][:],
                        in_=Z[:, :G * 676].to_broadcast((128, G * 676)))
            nc.gpsimd.dma_start(out=act2[:],
                                in_=Z[:, :BPC * 196].to_broadcast((128, BPC * 196)))

        with contextlib.ExitStack() as cctx:
            ps1 = cctx.enter_context(tc.tile_pool(name="ps1", bufs=2, space="PSUM"))
            ps2 = cctx.enter_context(tc.tile_pool(name="ps2", bufs=2, space="PSUM"))

            # conv1 chunking: pool-aligned column chunks of one image-half
            # (20 rows, 20 rows, 8 rows of the 48x48 raw conv output)
            C1_CHUNKS = [(0, 960, 20), (960, 960, 20), (1920, 384, 8)]

            def conv1_chunk(g, ci, imt, h, c):
                """One conv1 chunk: full-128-mode matmuls (K=65, taps in rows
                0-8, zeros above), then relu-first eviction + 2x2 pool."""
                lo, sz, rows = C1_CHUNKS[c]
                a1v = act1[g % 2][h][:].rearrange(
                    "p (b y x) -> p b y x", y=26, x=26)
                ps = ps1.tile([128, 1024], F32, tag="ps1", name="ps1")
                for j in range(0, sz, 512):
                    w = min(512, sz - j)
                    nc.tensor.matmul(
                        out=ps[:, j:j + w],
                        lhsT=w1t[0:73, 128 * h:128 * (h + 1)],
                        rhs=imt[0:73, 2304 * ci + lo + j:2304 * ci + lo + j + w],
                        start=True, stop=True)
                # relu+bias from PSUM -> bf16 (rows x 48)
                blk = e1p.tile([128, 960], DT_MM, tag="blk", name="blk")
                nc.scalar.activation(
                    out=blk[:, 0:sz], in_=ps[:, 0:sz],
                    func=RELU, bias=b1t[:, h:h + 1])
                bv = blk[:, 0:sz].rearrange("p (y x t) -> p y x t", x=24, t=2)
                sbx = e1p.tile([128, 480], DT_MM, tag="sbx", name="sbx")
                sxv = sbx[:, 0:sz // 2].rearrange("p (y x) -> p y x", x=24)
                nc.vector.tensor_max(sxv, bv[:, :, :, 0], bv[:, :, :, 1])
                sxp = sbx[:, 0:sz // 2].rearrange("p (y t x) -> p y t x",
                                                  t=2, x=24)
                # chunk c's pooled rows land at act1 interior rows
                # 1 + 10c .. + rows/2 (cols 1:25)
                dst = a1v[:, ci, 1 + 10 * c:1 + 10 * c + rows // 2, 1:25]
                nc.vector.tensor_max(
                    dst, sxp[:, :, 0, :], sxp[:, :, 1, :])

            def conv2_bursts(g, bb):
                """conv2 for image bb of group g as 6 sub-burst closures of
                6 matmuls each (2 psum groups of 18); the last also evicts."""
                a1vs = [act1[g % 2][h][:].rearrange(
                    "p (b y x) -> p b y x", y=26, x=26) for h in range(2)]
                a2v = act2[:].rearrange("p (b y x) -> p b y x", y=14, x=14)
                ps = ps2.tile([128, 1024], F32, tag="ps2", name="ps2")
                psr = ps[:].rearrange("p (rr c) -> p rr c", c=512)
                mms = []
                for rr in range(2):
                    for t in range(9):
                        dy, dx = divmod(t, 3)
                        for h in range(2):
                            mms.append((rr, t, h, dy, dx))

                def emit_mm(idx):
                    rr, t, h, dy, dx = mms[idx]
                    n = idx % 18
                    nc.tensor.matmul(
                        out=psr[:, rr, 0:288],
                        lhsT=w2t[:, (t * 2 + h) * 128:(t * 2 + h + 1) * 128],
                        rhs=a1vs[h][:, bb, 12 * rr + dy:12 * rr + dy + 12,
                                    dx:dx + 24],
                        start=(n == 0), stop=(n == 17))

                def evict():
                    sb2 = e2p.tile([128, 576], DT_MM, tag="sb2", name="sb2")
                    nc.scalar.activation(
                        out=sb2[:], in_=psr[:, :, 0:288],
                        func=RELU, bias=b2t[:])
                    s2v = sb2[:].rearrange("p (y x t) -> p y x t", x=12, t=2)
                    xm = e2p.tile([128, 288], DT_MM, tag="xm2", name="xm2")
                    xmv = xm[:].rearrange("p (y x) -> p y x", x=12)
                    nc.vector.tensor_max(xmv, s2v[:, :, :, 0], s2v[:, :, :, 1])
                    xmp = xm[:].rearrange("p (y t x) -> p y t x", t=2, x=12)
                    nc.vector.tensor_max(
                        a2v[:, G * g + bb, 1:13, 1:13],
                        xmp[:, :, 0, :], xmp[:, :, 1, :])

                def burst(j):
                    def run():
                        for idx in range(6 * j, 6 * j + 6):
                            emit_mm(idx)
                        if j == 5:
                            evict()
                    return run
                return [burst(j) for j in range(6)]

            # ---- group loop: conv1(g) chunks interleaved with conv2(g-1)
            # sub-bursts (the conv2 matmuls cover conv1's psum eviction
            # latency between chunks) ----
            prev_g = None
            for g in range(NG):
                if g > 0:
                    xpv = xpads[g % 2][:].rearrange("p (y x) -> p y x", x=50)
                    nc.sync.dma_start(
                        out=xpv[0:G, 1:49, 1:49],
                        in_=x[G * g:G * (g + 1), :].rearrange("b (y x) -> b y x", x=48))
                    imt = imts[g % 2]
                    issue_taps(imt, xpads[g % 2][:], g)
                else:
                    imt = imts[0]
                for ci in range(G):
                    bursts = (conv2_bursts(prev_g, ci)
                              if prev_g is not None else [None] * 6)
                    for j, (h, c) in enumerate(
                            [(h, c) for h in range(2) for c in range(3)]):
                        if bursts[j] is not None:
                            bursts[j]()
                        conv1_chunk(g, ci, imt, h, c)
                prev_g = g
            for ci in range(G):
                for b in conv2_bursts(prev_g, ci):
                    b()

        # ---- conv3 (act2 complete): 2-way column tiling ----
        with contextlib.ExitStack() as cctx:
            ps3 = cctx.enter_context(tc.tile_pool(name="ps3", bufs=3, space="PSUM"))
            psq = cctx.enter_context(tc.tile_pool(name="psq", bufs=1, space="PSUM"))
            pssc = cctx.enter_context(tc.tile_pool(name="pssc", bufs=1, space="PSUM"))
            psT = cctx.enter_context(tc.tile_pool(name="psT", bufs=1, space="PSUM"))

            a2v = act2[:].rearrange("p (b y x) -> p b y x", y=14, x=14)
            o3v = out3[:].rearrange("p (b hw) -> p b hw", hw=36)
            if C3_TILED:
                for t in range(16):  # rounds of 4 images (2 col-tiled pairs)
                    ps = ps3.tile([128, 288], F32, tag="ps3", name="ps3")
                    for j in range(2):
                        for k in range(9):
                            dy, dx = divmod(k, 3)
                            nc.tensor.matmul(
                                out=ps[64 * j:64 * (j + 1), :],
                                lhsT=w3t[:, 64 * k:64 * (k + 1)],
                                rhs=a2v[:, 4 * t + 2 * j:4 * t + 2 * j + 2,
                                        dy:dy + 12, dx:dx + 12],
                                start=(k == 0), stop=(k == 8),
                                tile_position=(0, 64 * j))
                    sb3 = e3p.tile([128, 288], DT_MM, tag="sb3", name="sb3")
                    nc.scalar.activation(out=sb3[:], in_=ps[:], func=RELU,
                                         bias=b3t[:])
                    s3v = sb3[:].rearrange("p (b y x t) -> p b y x t", b=2, x=6, t=2)
                    xm = e3p.tile([128, 144], DT_MM, tag="xm3", name="xm3")
                    xmv = xm[:].rearrange("p (b y x) -> p b y x", b=2, x=6)
                    nc.vector.tensor_max(xmv, s3v[:, :, :, :, 0], s3v[:, :, :, :, 1])
                    xmp = xm[:].rearrange("p (b y t x) -> p b y t x", b=2, t=2, x=6)
                    # top pair -> out3 directly; bottom pair pools into a
                    # staging tile on partitions 64-127, then a tiny SBUF->
                    # SBUF DMA moves it down (DVE lanes are fixed).
                    dst = o3v[:, 4 * t:4 * t + 2, :].rearrange(
                        "p b (y x) -> p b y x", x=6)
                    nc.vector.tensor_max(
                        dst, xmp[0:64, :, :, 0, :], xmp[0:64, :, :, 1, :])
                    stg = e3p.tile([128, 72], DT_MM, tag="stg3", name="stg3")
                    sgv = stg[:].rearrange("p (b y x) -> p b y x", b=2, x=6)
                    nc.vector.tensor_max(
                        sgv[64:128, :, :, :],
                        xmp[64:128, :, :, 0, :], xmp[64:128, :, :, 1, :])
                    nc.gpsimd.dma_start(
                        out=out3[:, 36 * (4 * t + 2):36 * (4 * t + 4)],
                        in_=stg[64:128, :])
            else:
                for t in range(32):  # one image pair per round
                    ps = ps3.tile([64, 288], F32, tag="ps3", name="ps3")
                    for k in range(9):
                        dy, dx = divmod(k, 3)
                        nc.tensor.matmul(
                            out=ps[:],
                            lhsT=w3t[:, 64 * k:64 * (k + 1)],
                            rhs=a2v[:, 2 * t:2 * t + 2, dy:dy + 12, dx:dx + 12],
                            start=(k == 0), stop=(k == 8))
                    sb3 = e3p.tile([64, 288], DT_MM, tag="sb3", name="sb3")
                    nc.scalar.activation(out=sb3[:], in_=ps[:], func=RELU,
                                         bias=b3t[0:64, :])
                    s3v = sb3[:].rearrange("p (b y x t) -> p b y x t", b=2, x=6, t=2)
                    xm = e3p.tile([64, 144], DT_MM, tag="xm3", name="xm3")
                    xmv = xm[:].rearrange("p (b y x) -> p b y x", b=2, x=6)
                    nc.vector.tensor_max(xmv, s3v[:, :, :, :, 0], s3v[:, :, :, :, 1])
                    xmp = xm[:].rearrange("p (b y t x) -> p b y t x", b=2, t=2, x=6)
                    dst = o3v[:, 2 * t:2 * t + 2, :].rearrange(
                        "p b (y x) -> p b y x", x=6)
                    nc.vector.tensor_max(
                        dst, xmp[:, :, :, 0, :], xmp[:, :, :, 1, :])

            # ---- q = W_eff @ out4 + b_eff : accumulate over hw ----
            psq_t = psq.tile([64, 64], F32)
            for hw in range(36):
                nc.tensor.matmul(
                    out=psq_t[:],
                    lhsT=wet[:, 64 * hw:64 * (hw + 1)],
                    rhs=out3[:, hw:2304:36],
                    start=(hw == 0), stop=(hw == 35))
            q_sb = ap_pool.tile([64, 64], DT_MM)
            nc.vector.tensor_scalar_add(q_sb[:], psq_t[:], bet[:])

            # ---- scores: per-image matmuls -> [36, 64] psum ----
            pssc_t = pssc.tile([36, 64], F32)
            for b in range(BPC):
                nc.tensor.matmul(
                    out=pssc_t[:, b:b + 1],
                    lhsT=out3[:, 36 * b:36 * (b + 1)],
                    rhs=q_sb[:, b:b + 1],
                    start=True, stop=True)
            sc_sb = ap_pool.tile([36, 64], F32)
            nc.vector.tensor_copy(sc_sb[:], pssc_t[:])
            psT_t = psT.tile([64, 36], F32)
            nc.tensor.transpose(psT_t[:], sc_sb[:], ident[:])

            # ---- softmax over hw (free dim) ----
            mx = ap_pool.tile([64, 1], F32)
            nc.vector.tensor_reduce(out=mx[:], in_=psT_t[:],
                                    op=mybir.AluOpType.max,
                                    axis=mybir.AxisListType.X)
            nmx = ap_pool.tile([64, 1], F32)
            nc.vector.tensor_scalar_mul(nmx[:], mx[:], -1.0)
            e_t = ap_pool.tile([64, 36], F32)
            nc.scalar.activation(out=e_t[:], in_=psT_t[:], func=EXP, bias=nmx[:])
            z = ap_pool.tile([64, 1], F32)
            nc.vector.tensor_reduce(out=z[:], in_=e_t[:],
                                    op=mybir.AluOpType.add,
                                    axis=mybir.AxisListType.X)
            rz = ap_pool.tile([64, 1], F32)
            nc.vector.reciprocal(rz[:], z[:])
            attn = ap_pool.tile([64, 36], DT_MM)
            nc.vector.tensor_scalar_mul(attn[:], e_t[:], rz[:])
            if debug:
                for pp in range(2):
                    for h in range(2):
                        nc.gpsimd.dma_start(out=dbg[f"dbg_act1_p{pp}h{h}"][:],
                                            in_=act1[pp][h][:])
                nc.gpsimd.dma_start(out=dbg["dbg_act2"][:], in_=act2[:])
                nc.gpsimd.dma_start(out=dbg["dbg_out3"][:], in_=out3[:])
                nc.gpsimd.dma_start(out=dbg["dbg_q"][:], in_=q_sb[:])
                nc.gpsimd.dma_start(out=dbg["dbg_attn"][:], in_=attn[:])
                nc.sync.dma_start(out=dbg["dbg_sc"][:], in_=sc_sb[:])

        # ---- g_mod + fc3 ----
        with contextlib.ExitStack() as cctx:
            psab = cctx.enter_context(tc.tile_pool(name="psab", bufs=1, space="PSUM"))
            psf = cctx.enter_context(tc.tile_pool(name="psf", bufs=1, space="PSUM"))

            attn_flat = ap_pool.tile([1, 2304], DT_MM)
            nc.sync.dma_start(out=attn_flat[:], in_=attn[:])
            psab_t = psab.tile([64, 2304], F32)
            for c in range(5):
                lo = 512 * c
                hi = min(lo + 512, 2304)
                nc.tensor.matmul(out=psab_t[:, lo:hi], lhsT=ones1[:],
                                 rhs=attn_flat[:, lo:hi], start=True, stop=True)
            # in-place: out3 is not needed after this product
            nc.vector.tensor_mul(out3[:], out3[:], psab_t[:])
            gT = ap_pool.tile([64, 64], DT_MM)
            with nc.allow_low_precision(reason="bf16 attn-weighted sum"):
                nc.vector.tensor_reduce(
                    out=gT[:], in_=out3[:].rearrange("p (b hw) -> p b hw", hw=36),
                    op=mybir.AluOpType.add, axis=mybir.AxisListType.X)

            if debug:
                nc.gpsimd.dma_start(out=dbg["dbg_gT"][:], in_=gT[:])
            psf_t = psf.tile([64, 7], F32)
            nc.tensor.matmul(out=psf_t[:], lhsT=gT[:],
                             rhs=w3f[:], start=True, stop=True)
            out_sb = ap_pool.tile([64, 7], F32)
            nc.vector.tensor_add(out_sb[:], psf_t[:], fc3b_t[:])
            nc.sync.dma_start(out=out[:], in_=out_sb[:])

    _split_excess_waits(nc)
    return nc


def kernel(**inputs):
    from concourse.bass_utils import run_bass_kernel_spmd

    w = _prep_weights({k: np.asarray(v, np.float32) for k, v in inputs.items()
                       if k != 'x'})
    npdt = mybir.dt.np(DT_MM)
    for k in ('W1T', 'W2T', 'W3T', 'WeT2', 'W3fT', 'Z', 'ONES'):
        w[k] = w[k].astype(npdt)
    xs = np.asarray(inputs['x'], np.float32).reshape(B_TOTAL, 2304).astype(npdt)

    nc = build_program()
    in_maps = []
    for c in range(N_CORES):
        m = {'x': np.ascontiguousarray(xs[BPC * c:BPC * (c + 1)])}
        m.update({k: v for k, v in w.items()})
        in_maps.append(m)
    res = run_bass_kernel_spmd(nc, in_maps, list(range(N_CORES)))
    outs = [res.results[c]['out'] for c in range(N_CORES)]
    return np.concatenate(outs, axis=0).astype(np.float32)


if __name__ == '__main__':
    rng = np.random.default_rng(0)
    fake = {
        'x': rng.standard_normal((512, 1, 48, 48), dtype=np.float32),
        'conv1_w': rng.standard_normal((256, 1, 3, 3), dtype=np.float32) * 0.05,
        'conv1_b': np.zeros(256, np.float32),
        'bn1_g': np.ones(256, np.float32), 'bn1_b': np.zeros(256, np.float32),
        'bn1_m': np.zeros(256, np.float32), 'bn1_v': np.ones(256, np.float32),
        'conv2_w': rng.standard_normal((128, 256, 3, 3), dtype=np.float32) * 0.05,
        'conv2_b': np.zeros(128, np.float32),
        'bn2_g': np.ones(128, np.float32), 'bn2_b': np.zeros(128, np.float32),
        'bn2_m': np.zeros(128, np.float32), 'bn2_v': np.ones(128, np.float32),
        'conv3_w': rng.standard_normal((64, 128, 3, 3), dtype=np.float32) * 0.05,
        'conv3_b': np.zeros(64, np.float32),
        'bn3_g': np.ones(64, np.float32), 'bn3_b': np.zeros(64, np.float32),
        'bn3_m': np.zeros(64, np.float32), 'bn3_v': np.ones(64, np.float32),
        'fc1_w': rng.standard_normal((512, 2304), dtype=np.float32) * 0.05,
        'fc1_b': np.zeros(512, np.float32),
        'fc2_w': rng.standard_normal((256, 512), dtype=np.float32) * 0.05,
        'fc2_b': np.zeros(256, np.float32),
        'att_w': rng.standard_normal((64, 256), dtype=np.float32) * 0.05,
        'att_b': np.zeros(64, np.float32),
        'fc3_w': rng.standard_normal((7, 64), dtype=np.float32) * 0.05,
        'fc3_b': np.zeros(7, np.float32),
    }
    print(kernel(**fake).shape)


# revision 34
# speedup vs baseline: 1.4098x; 1.0745x over previous
"""Trainium2 Bass kernel for nn_BaselineModel_80796924772520 (dense_cnn).

Self-contained: kernel(**inputs) -> np.ndarray [512, 7] float32.

Strategy: pure data parallelism over 8 NeuronCores (64 images each).
 - BN folded into conv weights/biases on host; fc1/fc2/att collapse into
   one linear map W_eff [64, 2304] on host.
 - conv1 (C_in=1, K=9): 4-way PE row tiling (tile_position=(32i,0), one
   9-row contraction per 32-row tile) -> 4 concurrent matmuls. im2col
   rows live at partitions 32i+k; no contraction zero-padding needed.
 - conv2: 9-tap border-clipped accumulating matmuls over UNPADDED bf16
   activations (center tap first covers the full region with start=True;
   shifted taps accumulate partial rectangles = zero-padding semantics).
 - conv3: same clipping + 2-way PE column tiling (two image pairs run
   concurrently at tile_position=(0,0)/(0,64)).
 - evictions: scalar ACT does relu+bias straight from PSUM into bf16
   SBUF (the only cheap PSUM reader), then 2x2 maxpool = two small DVE
   tensor_max ops at 2x mode (relu/bias commute with max).
 - attention: per-image [64x36]^T@[64x1] matmuls -> PE transpose ->
   softmax -> broadcast-matmul with ones -> multiply+segmented reduce.
"""
import sys
if '/opt/trn_rl_repo' not in sys.path:
    sys.path.insert(0, '/opt/trn_rl_repo')

import contextlib
import numpy as np

import concourse.bass as bass
import concourse.mybir as mybir
import concourse.tile as tile

F32 = mybir.dt.float32
BF16 = mybir.dt.bfloat16
DT_MM = BF16
RELU = mybir.ActivationFunctionType.Relu
EXP = mybir.ActivationFunctionType.Exp

N_CORES = 8
B_TOTAL = 512
BPC = B_TOTAL // N_CORES   # 64 images per core
G = 8                      # images per group
NG = BPC // G              # 8 groups
EPS = 1e-5

# switches
C3_TILED = False   # conv3 2-way PE column tiling
USE_MEMSET = False  # engine memsets for act border zeroing (else DMA bcast)

_MAX_WAITS = 1  # this walrus build supports 1 sync-wait per instruction


def _install_tile_fixups():
    """The nix walrus here allows only ONE sync-wait per instruction; Tile's
    exit drain aggregates one wait per live proc onto a single Drain. Spread
    the waits across spare SP nops emitted just before the drain."""
    if getattr(tile.TileContext, '_drain_patched', False):
        return

    def _patched(self, tick_clock, wait_clock):
        from concourse.vector_clock import ScopedClock
        nc = self.nc
        nops = [nc.sync.nop().ins for _ in range(32)]
        drain_inst = nc.sync.drain()
        wait_clock.add_sem_waits(
            drain_inst.ins, ScopedClock({None: tick_clock.global_clock}))
        si = drain_inst.ins.sync_info
        if si is not None and len(si.on_wait) > _MAX_WAITS:
            waits = list(si.on_wait)
            drain_inst.ins.sync_info = mybir.SyncInfo(
                on_wait=waits[:_MAX_WAITS], on_update=list(si.on_update))
            rest = waits[_MAX_WAITS:]
            for i in range(0, len(rest), _MAX_WAITS):
                nops[i // _MAX_WAITS].sync_info = mybir.SyncInfo(
                    on_wait=rest[i:i + _MAX_WAITS], on_update=[])
        nc.all_engine_barrier()
        popped = nc._tile_sem_poison_stack.pop()
        assert popped is self._sem_poison
        nc.clear_and_free_semaphores(list(self.sems.allocated().values()))
        nc.all_engine_barrier()

    tile.TileContext._drain_and_barrier = _patched
    tile.TileContext._drain_patched = True


def _split_excess_waits(nc):
    """This walrus allows one sync-wait per instruction. Hoist excess waits
    onto same-engine nops inserted immediately before the instruction
    (sequential waits on one engine are equivalent to a combined wait)."""
    idx = 0
    for f in nc.m.functions:
        for b in f.blocks:
            out, changed = [], False
            for ins in b.instructions:
                si = ins.sync_info
                if si is not None and len(si.on_wait) > _MAX_WAITS:
                    waits = list(si.on_wait)
                    extra, keep = waits[:-_MAX_WAITS], waits[-_MAX_WAITS:]
                    for j in range(0, len(extra), _MAX_WAITS):
                        nop = mybir.InstNoOp(name=f"I-wsplit-{idx}")
                        idx += 1
                        nop.engine = ins.engine
                        nop.sync_info = mybir.SyncInfo(
                            on_wait=extra[j:j + _MAX_WAITS], on_update=[])
                        nc.register_instruction(nop, overwrite=True)
                        out.append(nop)
                    ins.sync_info = mybir.SyncInfo(
                        on_wait=keep, on_update=list(si.on_update))
                    changed = True
                out.append(ins)
            if changed:
                b.instructions = out


def _prep_weights(p):
    """Fold BN, collapse FC chain, lay out weights for the device program."""
    def fold(w, b, g, be, m, v):
        inv = (g / np.sqrt(v + EPS)).astype(np.float32)
        wf = (w * inv[:, None, None, None]).astype(np.float32)
        bf = ((b - m) * inv + be).astype(np.float32)
        return wf, bf

    w1, b1 = fold(p['conv1_w'], p['conv1_b'], p['bn1_g'], p['bn1_b'], p['bn1_m'], p['bn1_v'])
    w2, b2 = fold(p['conv2_w'], p['conv2_b'], p['bn2_g'], p['bn2_b'], p['bn2_m'], p['bn2_v'])
    w3, b3 = fold(p['conv3_w'], p['conv3_b'], p['bn3_g'], p['bn3_b'], p['bn3_m'], p['bn3_v'])

    # conv1 lhsT [128, 256]: rows 64+k (k = 3*ky+kx) = w1[c, 0, ky, kx];
    # rows 0-63 stay zero (K padded to 73 keeps the PE in full 128-mode,
    # and the zero region [0:64] is quadrant-aligned for engine memsets
    # while staying disjoint from the tap-DMA rows 64-72).
    W1T = np.zeros((128, 256), np.float32)
    W1T[64:73, :] = w1.reshape(256, 9).T
    # conv2 lhsT [128, 2304]: [p, t*256 + h*128 + m] = w2[m, 128h+p, t]
    W2T = np.ascontiguousarray(
        w2.reshape(128, 2, 128, 9).transpose(2, 3, 1, 0)  # [p, t, h, m]
    ).reshape(128, 2304)
    # conv3 lhsT [128, 576]: [p, t*64 + m] = w3[m, p, t]
    W3T = np.ascontiguousarray(
        w3.reshape(64, 128, 9).transpose(1, 2, 0)).reshape(128, 576)

    # FC chain collapse: q = out4 @ W_eff.T + b_eff
    fc1w, fc2w, attw = p['fc1_w'], p['fc2_w'], p['att_w']
    W_eff = (attw @ fc2w @ fc1w).astype(np.float32)          # [64, 2304]
    b_eff = (attw @ (fc2w @ p['fc1_b'] + p['fc2_b']) + p['att_b']).astype(np.float32)
    # WeT2 [64, 2304]: [c, hw*64 + m] = W_eff[m, c*36 + hw]
    WeT2 = np.ascontiguousarray(
        W_eff.reshape(64, 64, 36).transpose(1, 2, 0)).reshape(64, 2304)

    W3fT = np.ascontiguousarray(p['fc3_w'].T).astype(np.float32)  # [64, 7]
    fc3b_rep = np.broadcast_to(p['fc3_b'], (64, 7)).astype(np.float32).copy()

    b1c = np.ascontiguousarray(b1.reshape(2, 128).T)       # [128, 2]
    b2c = b2.reshape(128, 1).astype(np.float32)
    b3c = np.concatenate([b3, b3]).reshape(128, 1).astype(np.float32)
    beffc = b_eff.reshape(64, 1).astype(np.float32)

    return dict(W1T=W1T, W2T=W2T, W3T=W3T, WeT2=WeT2, W3fT=W3fT,
                fc3b_rep=fc3b_rep, b1c=b1c, b2c=b2c, b3c=b3c, beffc=beffc,
                Z=np.zeros((1, 18432), np.float32),
                IDENT=np.eye(36, dtype=np.float32),
                ONES=np.ones((1, 64), np.float32))


def build_program(debug=False):
    """Build the per-core SPMD Bass program. Returns nc."""
    _install_tile_fixups()
    nc = bass.Bass("TRN2", target_bir_lowering=False, debug=False)

    x = nc.declare_dram_parameter("x", [BPC, 2304], DT_MM, isOutput=False)
    W1T = nc.declare_dram_parameter("W1T", [128, 256], DT_MM, isOutput=False)
    W2T = nc.declare_dram_parameter("W2T", [128, 2304], DT_MM, isOutput=False)
    W3T = nc.declare_dram_parameter("W3T", [128, 576], DT_MM, isOutput=False)
    WeT2 = nc.declare_dram_parameter("WeT2", [64, 2304], DT_MM, isOutput=False)
    W3fT = nc.declare_dram_parameter("W3fT", [64, 7], DT_MM, isOutput=False)
    fc3b = nc.declare_dram_parameter("fc3b_rep", [64, 7], F32, isOutput=False)
    b1c = nc.declare_dram_parameter("b1c", [128, 2], F32, isOutput=False)
    b2c = nc.declare_dram_parameter("b2c", [128, 1], F32, isOutput=False)
    b3c = nc.declare_dram_parameter("b3c", [128, 1], F32, isOutput=False)
    beffc = nc.declare_dram_parameter("beffc", [64, 1], F32, isOutput=False)
    Z = nc.declare_dram_parameter("Z", [1, 18432], DT_MM, isOutput=False)
    IDENT = nc.declare_dram_parameter("IDENT", [36, 36], F32, isOutput=False)
    ONES = nc.declare_dram_parameter("ONES", [1, 64], DT_MM, isOutput=False)
    out = nc.declare_dram_parameter("out", [BPC, 7], F32, isOutput=True)
    dbg = {}
    if debug:
        for nm, shp in [("dbg_act1_p0h0", [128, G * 676]), ("dbg_act1_p0h1", [128, G * 676]),
                        ("dbg_act1_p1h0", [128, G * 676]), ("dbg_act1_p1h1", [128, G * 676]),
                        ("dbg_act2", [128, BPC * 196]), ("dbg_out3", [64, BPC * 36]),
                        ("dbg_q", [64, 64]), ("dbg_attn", [64, 36]),
                        ("dbg_gT", [64, 64]), ("dbg_sc", [36, 64])]:
            dbg[nm] = nc.declare_dram_parameter(nm, shp, F32, isOutput=True)

    with tile.TileContext(nc) as tc, contextlib.ExitStack() as ctx:
        wp = ctx.enter_context(tc.tile_pool(name="weights", bufs=1))
        ap_pool = ctx.enter_context(tc.tile_pool(name="acts", bufs=1))
        cp = ctx.enter_context(tc.tile_pool(name="im2col", bufs=2))
        e1p = ctx.enter_context(tc.tile_pool(name="ev1", bufs=4))
        e2p = ctx.enter_context(tc.tile_pool(name="ev2", bufs=4))
        e3p = ctx.enter_context(tc.tile_pool(name="ev3", bufs=4))

        # ---- prologue: xpad zero-fill + group-0 x + taps ahead of the
        # bulk weight DMAs ----
        xpads = [ap_pool.tile([8, 2500], DT_MM, tag=f"xpad{pp}",
                              name=f"xpad{pp}") for pp in range(2)]
        for pp in range(2):
            nc.gpsimd.dma_start(out=xpads[pp][:],
                                in_=Z[:, :2500].to_broadcast((8, 2500)))
        xpv0 = xpads[0][:].rearrange("p (y x) -> p y x", x=50)
        nc.sync.dma_start(
            out=xpv0[0:G, 1:49, 1:49],
            in_=x[0:G, :].rearrange("b (y x) -> b y x", x=48))

        w1t = wp.tile([128, 256], DT_MM)
        nc.sync.dma_start(out=w1t[:], in_=W1T[:])
        b1t = wp.tile([128, 2], F32)
        nc.sync.dma_start(out=b1t[:], in_=b1c[:])

        # two persistent im2col slots; rows 0-63 zeroed once here (the
        # taps only ever rewrite rows 64-72; K padded to 73 keeps the PE
        # in full 128-mode)
        imts = [ap_pool.tile([73, G * 2304], DT_MM, tag=f"imts{j}",
                             name=f"imts{j}") for j in range(2)]
        nc.vector.memset(imts[0][0:64, :], 0.0)
        nc.gpsimd.memset(imts[1][0:64, :], 0.0)

        def issue_taps(imt, xpv, g):
            """9 DMAs: tap k -> imt row 64+k; cols = (img, y(48), x(48))."""
            for k in range(9):
                dy, dx = divmod(k, 3)
                eng = (nc.sync, nc.gpsimd)[k % 2]
                src = xpv[0:G, :].rearrange("p (y x) -> p y x", x=50)
                eng.dma_start(
                    out=imt[64 + k:64 + k + 1, :],
                    in_=src[:, dy:dy + 48, dx:dx + 48])
            return imt

        # group-0 im2col ahead of heavy weight loads
        issue_taps(imts[0], xpads[0][:], 0)

        # ---- remaining weights ----
        w2t = wp.tile([128, 2304], DT_MM)
        nc.sync.dma_start(out=w2t[:], in_=W2T[:])
        w3t = wp.tile([128, 576], DT_MM)
        nc.sync.dma_start(out=w3t[:], in_=W3T[:])
        wet = wp.tile([64, 2304], DT_MM)
        nc.sync.dma_start(out=wet[:], in_=WeT2[:])
        w3f = wp.tile([64, 7], DT_MM)
        nc.sync.dma_start(out=w3f[:], in_=W3fT[:])
        fc3b_t = wp.tile([64, 7], F32)
        nc.sync.dma_start(out=fc3b_t[:], in_=fc3b[:])
        b2t = wp.tile([128, 1], F32)
        nc.sync.dma_start(out=b2t[:], in_=b2c[:])
        b3t = wp.tile([128, 1], F32)
        nc.sync.dma_start(out=b3t[:], in_=b3c[:])
        bet = wp.tile([64, 1], F32)
        nc.sync.dma_start(out=bet[:], in_=beffc[:])
        ident = wp.tile([36, 36], F32)
        nc.sync.dma_start(out=ident[:], in_=IDENT[:])
        ones1 = wp.tile([1, 64], DT_MM)
        nc.sync.dma_start(out=ones1[:], in_=ONES[:])

        # ---- persistent activation buffers (zero-padded interiors; the
        # borders are zeroed ONCE here via engine memsets on otherwise-idle
        # engines — interior writes never touch them) ----
        act1 = [[ap_pool.tile([128, G * 676], DT_MM, tag=f"act1_{pp}_{h}",
                              name=f"act1_{pp}_{h}") for h in range(2)]
                for pp in range(2)]
        act2 = ap_pool.tile([128, BPC * 196], DT_MM)
        out3 = ap_pool.tile([64, BPC * 36], DT_MM)
        if USE_MEMSET:
            for pp in range(2):
                for h in range(2):
                    nc.vector.memset(act1[pp][h][:], 0.0)
            nc.gpsimd.memset(act2[:], 0.0)
        else:
            for pp in range(2):
                for h in range(2):
                    nc.sync.dma_start(
                        out=act1[pp][h][:],
                        in_=Z[:, :G * 676].to_broadcast((128, G * 676)))
            nc.gpsimd.dma_start(out=act2[:],
                                in_=Z[:, :BPC * 196].to_broadcast((128, BPC * 196)))

        with contextlib.ExitStack() as cctx:
            ps1 = cctx.enter_context(tc.tile_pool(name="ps1", bufs=2, space="PSUM"))
            ps2 = cctx.enter_context(tc.tile_pool(name="ps2", bufs=2, space="PSUM"))

            # conv1 chunking: pool-aligned column chunks of one image-half
            # (20 rows, 20 rows, 8 rows of the 48x48 raw conv output)
            C1_CHUNKS = [(0, 960, 20), (960, 960, 20), (1920, 384, 8)]

            def conv1_chunk(g, ci, imt, h, c):
                """One conv1 chunk: full-128-mode matmuls (K=65, taps in rows
                0-8, zeros above), then relu-first eviction + 2x2 pool."""
                lo, sz, rows = C1_CHUNKS[c]
                a1v = act1[g % 2][h][:].rearrange(
                    "p (b y x) -> p b y x", y=26, x=26)
                ps = ps1.tile([128, 1024], F32, tag="ps1", name="ps1")
                for j in range(0, sz, 512):
                    w = min(512, sz - j)
                    nc.tensor.matmul(
                        out=ps[:, j:j + w],
                        lhsT=w1t[0:73, 128 * h:128 * (h + 1)],
                        rhs=imt[0:73, 2304 * ci + lo + j:2304 * ci + lo + j + w],
                        start=True, stop=True)
                # relu+bias from PSUM -> bf16 (rows x 48)
                blk = e1p.tile([128, 960], DT_MM, tag="blk", name="blk")
                nc.scalar.activation(
                    out=blk[:, 0:sz], in_=ps[:, 0:sz],
                    func=RELU, bias=b1t[:, h:h + 1])
                bv = blk[:, 0:sz].rearrange("p (y x t) -> p y x t", x=24, t=2)
                sbx = e1p.tile([128, 480], DT_MM, tag="sbx", name="sbx")
                sxv = sbx[:, 0:sz // 2].rearrange("p (y x) -> p y x", x=24)
                nc.vector.tensor_max(sxv, bv[:, :, :, 0], bv[:, :, :, 1])
                sxp = sbx[:, 0:sz // 2].rearrange("p (y t x) -> p y t x",
                                                  t=2, x=24)
                # chunk c's pooled rows land at act1 interior rows
                # 1 + 10c .. + rows/2 (cols 1:25)
                dst = a1v[:, ci, 1 + 10 * c:1 + 10 * c + rows // 2, 1:25]
                nc.vector.tensor_max(
                    dst, sxp[:, :, 0, :], sxp[:, :, 1, :])

            def conv2_bursts(g, bb):
                """conv2 for image bb of group g as 6 sub-burst closures of
                6 matmuls each (2 psum groups of 18); the last also evicts."""
                a1vs = [act1[g % 2][h][:].rearrange(
                    "p (b y x) -> p b y x", y=26, x=26) for h in range(2)]
                a2v = act2[:].rearrange("p (b y x) -> p b y x", y=14, x=14)
                ps = ps2.tile([128, 1024], F32, tag="ps2", name="ps2")
                psr = ps[:].rearrange("p (rr c) -> p rr c", c=512)
                mms = []
                for rr in range(2):
                    for t in range(9):
                        dy, dx = divmod(t, 3)
                        for h in range(2):
                            mms.append((rr, t, h, dy, dx))

                def emit_mm(idx):
                    rr, t, h, dy, dx = mms[idx]
                    n = idx % 18
                    nc.tensor.matmul(
                        out=psr[:, rr, 0:288],
                        lhsT=w2t[:, (t * 2 + h) * 128:(t * 2 + h + 1) * 128],
                        rhs=a1vs[h][:, bb, 12 * rr + dy:12 * rr + dy + 12,
                                    dx:dx + 24],
                        start=(n == 0), stop=(n == 17))

                def evict():
                    sb2 = e2p.tile([128, 576], DT_MM, tag="sb2", name="sb2")
                    nc.scalar.activation(
                        out=sb2[:], in_=psr[:, :, 0:288],
                        func=RELU, bias=b2t[:])
                    s2v = sb2[:].rearrange("p (y x t) -> p y x t", x=12, t=2)
                    xm = e2p.tile([128, 288], DT_MM, tag="xm2", name="xm2")
                    xmv = xm[:].rearrange("p (y x) -> p y x", x=12)
                    nc.vector.tensor_max(xmv, s2v[:, :, :, 0], s2v[:, :, :, 1])
                    xmp = xm[:].rearrange("p (y t x) -> p y t x", t=2, x=12)
                    nc.vector.tensor_max(
                        a2v[:, G * g + bb, 1:13, 1:13],
                        xmp[:, :, 0, :], xmp[:, :, 1, :])

                def burst(j):
                    def run():
                        for idx in range(6 * j, 6 * j + 6):
                            emit_mm(idx)
                        if j == 5:
                            evict()
                    return run
                return [burst(j) for j in range(6)]

            # ---- group loop: conv1(g) chunks interleaved with conv2(g-1)
            # sub-bursts (the conv2 matmuls cover conv1's psum eviction
            # latency between chunks) ----
            prev_g = None
            for g in range(NG):
                if g > 0:
                    xpv = xpads[g % 2][:].rearrange("p (y x) -> p y x", x=50)
                    nc.sync.dma_start(
                        out=xpv[0:G, 1:49, 1:49],
                        in_=x[G * g:G * (g + 1), :].rearrange("b (y x) -> b y x", x=48))
                    imt = imts[g % 2]
                    issue_taps(imt, xpads[g % 2][:], g)
                else:
                    imt = imts[0]
                for ci in range(G):
                    bursts = (conv2_bursts(prev_g, ci)
                              if prev_g is not None else [None] * 6)
                    for j, (h, c) in enumerate(
                            [(h, c) for h in range(2) for c in range(3)]):
                        if bursts[j] is not None:
                            bursts[j]()
                        conv1_chunk(g, ci, imt, h, c)
                prev_g = g
            for ci in range(G):
                for b in conv2_bursts(prev_g, ci):
                    b()

        # ---- conv3 (act2 complete): 2-way column tiling ----
        with contextlib.ExitStack() as cctx:
            ps3 = cctx.enter_context(tc.tile_pool(name="ps3", bufs=3, space="PSUM"))
            psq = cctx.enter_context(tc.tile_pool(name="psq", bufs=1, space="PSUM"))
            pssc = cctx.enter_context(tc.tile_pool(name="pssc", bufs=1, space="PSUM"))
            psT = cctx.enter_context(tc.tile_pool(name="psT", bufs=1, space="PSUM"))

            a2v = act2[:].rearrange("p (b y x) -> p b y x", y=14, x=14)
            o3v = out3[:].rearrange("p (b hw) -> p b hw", hw=36)
            if C3_TILED:
                for t in range(16):  # rounds of 4 images (2 col-tiled pairs)
                    ps = ps3.tile([128, 288], F32, tag="ps3", name="ps3")
                    for j in range(2):
                        for k in range(9):
                            dy, dx = divmod(k, 3)
                            nc.tensor.matmul(
                                out=ps[64 * j:64 * (j + 1), :],
                                lhsT=w3t[:, 64 * k:64 * (k + 1)],
                                rhs=a2v[:, 4 * t + 2 * j:4 * t + 2 * j + 2,
                                        dy:dy + 12, dx:dx + 12],
                                start=(k == 0), stop=(k == 8),
                                tile_position=(0, 64 * j))
                    sb3 = e3p.tile([128, 288], DT_MM, tag="sb3", name="sb3")
                    nc.scalar.activation(out=sb3[:], in_=ps[:], func=RELU,
                                         bias=b3t[:])
                    s3v = sb3[:].rearrange("p (b y x t) -> p b y x t", b=2, x=6, t=2)
                    xm = e3p.tile([128, 144], DT_MM, tag="xm3", name="xm3")
                    xmv = xm[:].rearrange("p (b y x) -> p b y x", b=2, x=6)
                    nc.vector.tensor_max(xmv, s3v[:, :, :, :, 0], s3v[:, :, :, :, 1])
                    xmp = xm[:].rearrange("p (b y t x) -> p b y t x", b=2, t=2, x=6)
                    # top pair -> out3 directly; bottom pair pools into a
                    # staging tile on partitions 64-127, then a tiny SBUF->
                    # SBUF DMA moves it down (DVE lanes are fixed).
                    dst = o3v[:, 4 * t:4 * t + 2, :].rearrange(
                        "p b (y x) -> p b y x", x=6)
                    nc.vector.tensor_max(
                        dst, xmp[0:64, :, :, 0, :], xmp[0:64, :, :, 1, :])
                    stg = e3p.tile([128, 72], DT_MM, tag="stg3", name="stg3")
                    sgv = stg[:].rearrange("p (b y x) -> p b y x", b=2, x=6)
                    nc.vector.tensor_max(
                        sgv[64:128, :, :, :],
                        xmp[64:128, :, :, 0, :], xmp[64:128, :, :, 1, :])
                    nc.gpsimd.dma_start(
                        out=out3[:, 36 * (4 * t + 2):36 * (4 * t + 4)],
                        in_=stg[64:128, :])
            else:
                for t in range(32):  # one image pair per round
                    ps = ps3.tile([64, 288], F32, tag="ps3", name="ps3")
                    for k in range(9):
                        dy, dx = divmod(k, 3)
                        nc.tensor.matmul(
                            out=ps[:],
                            lhsT=w3t[:, 64 * k:64 * (k + 1)],
                            rhs=a2v[:, 2 * t:2 * t + 2, dy:dy + 12, dx:dx + 12],
                            start=(k == 0), stop=(k == 8))
                    sb3 = e3p.tile([64, 288], DT_MM, tag="sb3", name="sb3")
                    nc.scalar.activation(out=sb3[:], in_=ps[:], func=RELU,
                                         bias=b3t[0:64, :])
                    s3v = sb3[:].rearrange("p (b y x t) -> p b y x t", b=2, x=6, t=2)
                    xm = e3p.tile([64, 144], DT_MM, tag="xm3", name="xm3")
                    xmv = xm[:].rearrange("p (b y x) -> p b y x", b=2, x=6)
                    nc.vector.tensor_max(xmv, s3v[:, :, :, :, 0], s3v[:, :, :, :, 1])
                    xmp = xm[:].rearrange("p (b y t x) -> p b y t x", b=2, t=2, x=6)
                    dst = o3v[:, 2 * t:2 * t + 2, :].rearrange(
                        "p b (y x) -> p b y x", x=6)
                    nc.vector.tensor_max(
                        dst, xmp[:, :, :, 0, :], xmp[:, :, :, 1, :])

            # ---- q = W_eff @ out4 + b_eff : accumulate over hw ----
            psq_t = psq.tile([64, 64], F32)
            for hw in range(36):
                nc.tensor.matmul(
                    out=psq_t[:],
                    lhsT=wet[:, 64 * hw:64 * (hw + 1)],
                    rhs=out3[:, hw:2304:36],
                    start=(hw == 0), stop=(hw == 35))
            q_sb = ap_pool.tile([64, 64], DT_MM)
            nc.vector.tensor_scalar_add(q_sb[:], psq_t[:], bet[:])

            # ---- scores: per-image matmuls -> [36, 64] psum ----
            pssc_t = pssc.tile([36, 64], F32)
            for b in range(BPC):
                nc.tensor.matmul(
                    out=pssc_t[:, b:b + 1],
                    lhsT=out3[:, 36 * b:36 * (b + 1)],
                    rhs=q_sb[:, b:b + 1],
                    start=True, stop=True)
            sc_sb = ap_pool.tile([36, 64], F32)
            nc.vector.tensor_copy(sc_sb[:], pssc_t[:])
            psT_t = psT.tile([64, 36], F32)
            nc.tensor.transpose(psT_t[:], sc_sb[:], ident[:])

            # ---- softmax over hw (free dim) ----
            mx = ap_pool.tile([64, 1], F32)
            nc.vector.tensor_reduce(out=mx[:], in_=psT_t[:],
                                    op=mybir.AluOpType.max,
                                    axis=mybir.AxisListType.X)
            nmx = ap_pool.tile([64, 1], F32)
            nc.vector.tensor_scalar_mul(nmx[:], mx[:], -1.0)
            e_t = ap_pool.tile([64, 36], F32)
            nc.scalar.activation(out=e_t[:], in_=psT_t[:], func=EXP, bias=nmx[:])
            z = ap_pool.tile([64, 1], F32)
            nc.vector.tensor_reduce(out=z[:], in_=e_t[:],
                                    op=mybir.AluOpType.add,
                                    axis=mybir.AxisListType.X)
            rz = ap_pool.tile([64, 1], F32)
            nc.vector.reciprocal(rz[:], z[:])
            attn = ap_pool.tile([64, 36], DT_MM)
            nc.vector.tensor_scalar_mul(attn[:], e_t[:], rz[:])
            if debug:
                for pp in range(2):
                    for h in range(2):
                        nc.gpsimd.dma_start(out=dbg[f"dbg_act1_p{pp}h{h}"][:],
                                            in_=act1[pp][h][:])
                nc.gpsimd.dma_start(out=dbg["dbg_act2"][:], in_=act2[:])
                nc.gpsimd.dma_start(out=dbg["dbg_out3"][:], in_=out3[:])
                nc.gpsimd.dma_start(out=dbg["dbg_q"][:], in_=q_sb[:])
                nc.gpsimd.dma_start(out=dbg["dbg_attn"][:], in_=attn[:])
                nc.sync.dma_start(out=dbg["dbg_sc"][:], in_=sc_sb[:])

        # ---- g_mod + fc3 ----
        with contextlib.ExitStack() as cctx:
            psab = cctx.enter_context(tc.tile_pool(name="psab", bufs=1, space="PSUM"))
            psf = cctx.enter_context(tc.tile_pool(name="psf", bufs=1, space="PSUM"))

            attn_flat = ap_pool.tile([1, 2304], DT_MM)
            nc.sync.dma_start(out=attn_flat[:], in_=attn[:])
            psab_t = psab.tile([64, 2304], F32)
            for c in range(5):
                lo = 512 * c
                hi = min(lo + 512, 2304)
                nc.tensor.matmul(out=psab_t[:, lo:hi], lhsT=ones1[:],
                                 rhs=attn_flat[:, lo:hi], start=True, stop=True)
            # in-place: out3 is not needed after this product
            nc.vector.tensor_mul(out3[:], out3[:], psab_t[:])
            gT = ap_pool.tile([64, 64], DT_MM)
            with nc.allow_low_precision(reason="bf16 attn-weighted sum"):
                nc.vector.tensor_reduce(
                    out=gT[:], in_=out3[:].rearrange("p (b hw) -> p b hw", hw=36),
                    op=mybir.AluOpType.add, axis=mybir.AxisListType.X)

            if debug:
                nc.gpsimd.dma_start(out=dbg["dbg_gT"][:], in_=gT[:])
            psf_t = psf.tile([64, 7], F32)
            nc.tensor.matmul(out=psf_t[:], lhsT=gT[:],
                             rhs=w3f[:], start=True, stop=True)
            out_sb = ap_pool.tile([64, 7], F32)
            nc.vector.tensor_add(out_sb[:], psf_t[:], fc3b_t[:])
            nc.sync.dma_start(out=out[:], in_=out_sb[:])

    _split_excess_waits(nc)
    return nc


def kernel(**inputs):
    from concourse.bass_utils import run_bass_kernel_spmd

    w = _prep_weights({k: np.asarray(v, np.float32) for k, v in inputs.items()
                       if k != 'x'})
    npdt = mybir.dt.np(DT_MM)
    for k in ('W1T', 'W2T', 'W3T', 'WeT2', 'W3fT', 'Z', 'ONES'):
        w[k] = w[k].astype(npdt)
    xs = np.asarray(inputs['x'], np.float32).reshape(B_TOTAL, 2304).astype(npdt)

    nc = build_program()
    in_maps = []
    for c in range(N_CORES):
        m = {'x': np.ascontiguousarray(xs[BPC * c:BPC * (c + 1)])}
        m.update({k: v for k, v in w.items()})
        in_maps.append(m)
    res = run_bass_kernel_spmd(nc, in_maps, list(range(N_CORES)))
    outs = [res.results[c]['out'] for c in range(N_CORES)]
    return np.concatenate(outs, axis=0).astype(np.float32)


if __name__ == '__main__':
    rng = np.random.default_rng(0)
    fake = {
        'x': rng.standard_normal((512, 1, 48, 48), dtype=np.float32),
        'conv1_w': rng.standard_normal((256, 1, 3, 3), dtype=np.float32) * 0.05,
        'conv1_b': np.zeros(256, np.float32),
        'bn1_g': np.ones(256, np.float32), 'bn1_b': np.zeros(256, np.float32),
        'bn1_m': np.zeros(256, np.float32), 'bn1_v': np.ones(256, np.float32),
        'conv2_w': rng.standard_normal((128, 256, 3, 3), dtype=np.float32) * 0.05,
        'conv2_b': np.zeros(128, np.float32),
        'bn2_g': np.ones(128, np.float32), 'bn2_b': np.zeros(128, np.float32),
        'bn2_m': np.zeros(128, np.float32), 'bn2_v': np.ones(128, np.float32),
        'conv3_w': rng.standard_normal((64, 128, 3, 3), dtype=np.float32) * 0.05,
        'conv3_b': np.zeros(64, np.float32),
        'bn3_g': np.ones(64, np.float32), 'bn3_b': np.zeros(64, np.float32),
        'bn3_m': np.zeros(64, np.float32), 'bn3_v': np.ones(64, np.float32),
        'fc1_w': rng.standard_normal((512, 2304), dtype=np.float32) * 0.05,
        'fc1_b': np.zeros(512, np.float32),
        'fc2_w': rng.standard_normal((256, 512), dtype=np.float32) * 0.05,
        'fc2_b': np.zeros(256, np.float32),
        'att_w': rng.standard_normal((64, 256), dtype=np.float32) * 0.05,
        'att_b': np.zeros(64, np.float32),
        'fc3_w': rng.standard_normal((7, 64), dtype=np.float32) * 0.05,
        'fc3_b': np.zeros(7, np.float32),
    }
    print(kernel(**fake).shape)


# revision 40
# speedup vs baseline: 1.4249x; 1.0107x over previous
"""Trainium2 Bass kernel for nn_BaselineModel_80796924772520 (dense_cnn).

Self-contained: kernel(**inputs) -> np.ndarray [512, 7] float32.

Strategy: pure data parallelism over 8 NeuronCores (64 images each).
 - BN folded into conv weights/biases on host; fc1/fc2/att collapse into
   one linear map W_eff [64, 2304] on host.
 - conv1 (C_in=1, K=9): 4-way PE row tiling (tile_position=(32i,0), one
   9-row contraction per 32-row tile) -> 4 concurrent matmuls. im2col
   rows live at partitions 32i+k; no contraction zero-padding needed.
 - conv2: 9-tap border-clipped accumulating matmuls over UNPADDED bf16
   activations (center tap first covers the full region with start=True;
   shifted taps accumulate partial rectangles = zero-padding semantics).
 - conv3: same clipping + 2-way PE column tiling (two image pairs run
   concurrently at tile_position=(0,0)/(0,64)).
 - evictions: scalar ACT does relu+bias straight from PSUM into bf16
   SBUF (the only cheap PSUM reader), then 2x2 maxpool = two small DVE
   tensor_max ops at 2x mode (relu/bias commute with max).
 - attention: per-image [64x36]^T@[64x1] matmuls -> PE transpose ->
   softmax -> broadcast-matmul with ones -> multiply+segmented reduce.
"""
import sys
if '/opt/trn_rl_repo' not in sys.path:
    sys.path.insert(0, '/opt/trn_rl_repo')

import contextlib
import numpy as np

import concourse.bass as bass
import concourse.mybir as mybir
import concourse.tile as tile

F32 = mybir.dt.float32
BF16 = mybir.dt.bfloat16
DT_MM = BF16
RELU = mybir.ActivationFunctionType.Relu
EXP = mybir.ActivationFunctionType.Exp

N_CORES = 8
B_TOTAL = 512
BPC = B_TOTAL // N_CORES   # 64 images per core
G = 8                      # images per group
NG = BPC // G              # 8 groups
EPS = 1e-5

# switches
C3_TILED = True    # conv3 2-way PE column tiling
USE_MEMSET = False  # engine memsets for act border zeroing (else DMA bcast)

_MAX_WAITS = 1  # this walrus build supports 1 sync-wait per instruction


def _install_tile_fixups():
    """The nix walrus here allows only ONE sync-wait per instruction; Tile's
    exit drain aggregates one wait per live proc onto a single Drain. Spread
    the waits across spare SP nops emitted just before the drain."""
    if getattr(tile.TileContext, '_drain_patched', False):
        return

    def _patched(self, tick_clock, wait_clock):
        from concourse.vector_clock import ScopedClock
        nc = self.nc
        nops = [nc.sync.nop().ins for _ in range(32)]
        drain_inst = nc.sync.drain()
        wait_clock.add_sem_waits(
            drain_inst.ins, ScopedClock({None: tick_clock.global_clock}))
        si = drain_inst.ins.sync_info
        if si is not None and len(si.on_wait) > _MAX_WAITS:
            waits = list(si.on_wait)
            drain_inst.ins.sync_info = mybir.SyncInfo(
                on_wait=waits[:_MAX_WAITS], on_update=list(si.on_update))
            rest = waits[_MAX_WAITS:]
            for i in range(0, len(rest), _MAX_WAITS):
                nops[i // _MAX_WAITS].sync_info = mybir.SyncInfo(
                    on_wait=rest[i:i + _MAX_WAITS], on_update=[])
        nc.all_engine_barrier()
        popped = nc._tile_sem_poison_stack.pop()
        assert popped is self._sem_poison
        nc.clear_and_free_semaphores(list(self.sems.allocated().values()))
        nc.all_engine_barrier()

    tile.TileContext._drain_and_barrier = _patched
    tile.TileContext._drain_patched = True


def _split_excess_waits(nc):
    """This walrus allows one sync-wait per instruction. Hoist excess waits
    onto same-engine nops inserted immediately before the instruction
    (sequential waits on one engine are equivalent to a combined wait)."""
    idx = 0
    for f in nc.m.functions:
        for b in f.blocks:
            out, changed = [], False
            for ins in b.instructions:
                si = ins.sync_info
                if si is not None and len(si.on_wait) > _MAX_WAITS:
                    waits = list(si.on_wait)
                    extra, keep = waits[:-_MAX_WAITS], waits[-_MAX_WAITS:]
                    for j in range(0, len(extra), _MAX_WAITS):
                        nop = mybir.InstNoOp(name=f"I-wsplit-{idx}")
                        idx += 1
                        nop.engine = ins.engine
                        nop.sync_info = mybir.SyncInfo(
                            on_wait=extra[j:j + _MAX_WAITS], on_update=[])
                        nc.register_instruction(nop, overwrite=True)
                        out.append(nop)
                    ins.sync_info = mybir.SyncInfo(
                        on_wait=keep, on_update=list(si.on_update))
                    changed = True
                out.append(ins)
            if changed:
                b.instructions = out


def _prep_weights(p):
    """Fold BN, collapse FC chain, lay out weights for the device program."""
    def fold(w, b, g, be, m, v):
        inv = (g / np.sqrt(v + EPS)).astype(np.float32)
        wf = (w * inv[:, None, None, None]).astype(np.float32)
        bf = ((b - m) * inv + be).astype(np.float32)
        return wf, bf

    w1, b1 = fold(p['conv1_w'], p['conv1_b'], p['bn1_g'], p['bn1_b'], p['bn1_m'], p['bn1_v'])
    w2, b2 = fold(p['conv2_w'], p['conv2_b'], p['bn2_g'], p['bn2_b'], p['bn2_m'], p['bn2_v'])
    w3, b3 = fold(p['conv3_w'], p['conv3_b'], p['bn3_g'], p['bn3_b'], p['bn3_m'], p['bn3_v'])

    # conv1 lhsT [128, 256]: rows 64+k (k = 3*ky+kx) = w1[c, 0, ky, kx];
    # rows 0-63 stay zero (K padded to 73 keeps the PE in full 128-mode,
    # and the zero region [0:64] is quadrant-aligned for engine memsets
    # while staying disjoint from the tap-DMA rows 64-72).
    W1T = np.zeros((128, 256), np.float32)
    W1T[64:73, :] = w1.reshape(256, 9).T
    # conv2 lhsT [128, 2304]: [p, t*256 + h*128 + m] = w2[m, 128h+p, t]
    W2T = np.ascontiguousarray(
        w2.reshape(128, 2, 128, 9).transpose(2, 3, 1, 0)  # [p, t, h, m]
    ).reshape(128, 2304)
    # conv3 lhsT [128, 576]: [p, t*64 + m] = w3[m, p, t]
    W3T = np.ascontiguousarray(
        w3.reshape(64, 128, 9).transpose(1, 2, 0)).reshape(128, 576)

    # FC chain collapse: q = out4 @ W_eff.T + b_eff
    fc1w, fc2w, attw = p['fc1_w'], p['fc2_w'], p['att_w']
    W_eff = (attw @ fc2w @ fc1w).astype(np.float32)          # [64, 2304]
    b_eff = (attw @ (fc2w @ p['fc1_b'] + p['fc2_b']) + p['att_b']).astype(np.float32)
    # WeT2 [64, 2304]: [c, hw*64 + m] = W_eff[m, c*36 + hw]
    WeT2 = np.ascontiguousarray(
        W_eff.reshape(64, 64, 36).transpose(1, 2, 0)).reshape(64, 2304)

    W3fT = np.ascontiguousarray(p['fc3_w'].T).astype(np.float32)  # [64, 7]
    fc3b_rep = np.broadcast_to(p['fc3_b'], (64, 7)).astype(np.float32).copy()

    b1c = np.ascontiguousarray(b1.reshape(2, 128).T)       # [128, 2]
    b2c = b2.reshape(128, 1).astype(np.float32)
    b3c = np.concatenate([b3, b3]).reshape(128, 1).astype(np.float32)
    beffc = b_eff.reshape(64, 1).astype(np.float32)

    return dict(W1T=W1T, W2T=W2T, W3T=W3T, WeT2=WeT2, W3fT=W3fT,
                fc3b_rep=fc3b_rep, b1c=b1c, b2c=b2c, b3c=b3c, beffc=beffc,
                Z=np.zeros((1, 18432), np.float32),
                IDENT=np.eye(36, dtype=np.float32),
                ONES=np.ones((1, 64), np.float32))


def build_program(debug=False):
    """Build the per-core SPMD Bass program. Returns nc."""
    _install_tile_fixups()
    nc = bass.Bass("TRN2", target_bir_lowering=False, debug=False)

    x = nc.declare_dram_parameter("x", [BPC, 2304], DT_MM, isOutput=False)
    W1T = nc.declare_dram_parameter("W1T", [128, 256], DT_MM, isOutput=False)
    W2T = nc.declare_dram_parameter("W2T", [128, 2304], DT_MM, isOutput=False)
    W3T = nc.declare_dram_parameter("W3T", [128, 576], DT_MM, isOutput=False)
    WeT2 = nc.declare_dram_parameter("WeT2", [64, 2304], DT_MM, isOutput=False)
    W3fT = nc.declare_dram_parameter("W3fT", [64, 7], DT_MM, isOutput=False)
    fc3b = nc.declare_dram_parameter("fc3b_rep", [64, 7], F32, isOutput=False)
    b1c = nc.declare_dram_parameter("b1c", [128, 2], F32, isOutput=False)
    b2c = nc.declare_dram_parameter("b2c", [128, 1], F32, isOutput=False)
    b3c = nc.declare_dram_parameter("b3c", [128, 1], F32, isOutput=False)
    beffc = nc.declare_dram_parameter("beffc", [64, 1], F32, isOutput=False)
    Z = nc.declare_dram_parameter("Z", [1, 18432], DT_MM, isOutput=False)
    IDENT = nc.declare_dram_parameter("IDENT", [36, 36], F32, isOutput=False)
    ONES = nc.declare_dram_parameter("ONES", [1, 64], DT_MM, isOutput=False)
    out = nc.declare_dram_parameter("out", [BPC, 7], F32, isOutput=True)
    dbg = {}
    if debug:
        for nm, shp in [("dbg_act1_p0h0", [128, G * 676]), ("dbg_act1_p0h1", [128, G * 676]),
                        ("dbg_act1_p1h0", [128, G * 676]), ("dbg_act1_p1h1", [128, G * 676]),
                        ("dbg_act2", [128, BPC * 196]), ("dbg_out3", [64, BPC * 36]),
                        ("dbg_q", [64, 64]), ("dbg_attn", [64, 36]),
                        ("dbg_gT", [64, 64]), ("dbg_sc", [36, 64])]:
            dbg[nm] = nc.declare_dram_parameter(nm, shp, F32, isOutput=True)

    with tile.TileContext(nc) as tc, contextlib.ExitStack() as ctx:
        wp = ctx.enter_context(tc.tile_pool(name="weights", bufs=1))
        ap_pool = ctx.enter_context(tc.tile_pool(name="acts", bufs=1))
        cp = ctx.enter_context(tc.tile_pool(name="im2col", bufs=2))
        e1p = ctx.enter_context(tc.tile_pool(name="ev1", bufs=4))
        e2p = ctx.enter_context(tc.tile_pool(name="ev2", bufs=4))
        e3p = ctx.enter_context(tc.tile_pool(name="ev3", bufs=4))

        # ---- prologue: xpad zero-fill + group-0 x + taps ahead of the
        # bulk weight DMAs ----
        xpads = [ap_pool.tile([8, 2500], DT_MM, tag=f"xpad{pp}",
                              name=f"xpad{pp}") for pp in range(2)]
        for pp in range(2):
            nc.gpsimd.dma_start(out=xpads[pp][:],
                                in_=Z[:, :2500].to_broadcast((8, 2500)))
        xpv0 = xpads[0][:].rearrange("p (y x) -> p y x", x=50)
        nc.sync.dma_start(
            out=xpv0[0:G, 1:49, 1:49],
            in_=x[0:G, :].rearrange("b (y x) -> b y x", x=48))

        w1t = wp.tile([128, 256], DT_MM)
        nc.sync.dma_start(out=w1t[:], in_=W1T[:])
        b1t = wp.tile([128, 2], F32)
        nc.sync.dma_start(out=b1t[:], in_=b1c[:])

        # two persistent im2col slots; rows 0-63 zeroed once here (the
        # taps only ever rewrite rows 64-72; K padded to 73 keeps the PE
        # in full 128-mode)
        imts = [ap_pool.tile([73, G * 2304], DT_MM, tag=f"imts{j}",
                             name=f"imts{j}") for j in range(2)]
        nc.vector.memset(imts[0][0:64, :], 0.0)
        nc.gpsimd.memset(imts[1][0:64, :], 0.0)

        def issue_taps(imt, xpv, g):
            """9 DMAs: tap k -> imt row 64+k; cols = (img, y(48), x(48))."""
            for k in range(9):
                dy, dx = divmod(k, 3)
                eng = (nc.sync, nc.gpsimd)[k % 2]
                src = xpv[0:G, :].rearrange("p (y x) -> p y x", x=50)
                eng.dma_start(
                    out=imt[64 + k:64 + k + 1, :],
                    in_=src[:, dy:dy + 48, dx:dx + 48])
            return imt

        # group-0 im2col ahead of heavy weight loads
        issue_taps(imts[0], xpads[0][:], 0)

        # ---- remaining weights ----
        w2t = wp.tile([128, 2304], DT_MM)
        nc.sync.dma_start(out=w2t[:], in_=W2T[:])
        w3t = wp.tile([128, 576], DT_MM)
        nc.sync.dma_start(out=w3t[:], in_=W3T[:])
        wet = wp.tile([64, 2304], DT_MM)
        nc.sync.dma_start(out=wet[:], in_=WeT2[:])
        w3f = wp.tile([64, 7], DT_MM)
        nc.sync.dma_start(out=w3f[:], in_=W3fT[:])
        fc3b_t = wp.tile([64, 7], F32)
        nc.sync.dma_start(out=fc3b_t[:], in_=fc3b[:])
        b2t = wp.tile([128, 1], F32)
        nc.sync.dma_start(out=b2t[:], in_=b2c[:])
        b3t = wp.tile([128, 1], F32)
        nc.sync.dma_start(out=b3t[:], in_=b3c[:])
        bet = wp.tile([64, 1], F32)
        nc.sync.dma_start(out=bet[:], in_=beffc[:])
        ident = wp.tile([36, 36], F32)
        nc.sync.dma_start(out=ident[:], in_=IDENT[:])
        ones1 = wp.tile([1, 64], DT_MM)
        nc.sync.dma_start(out=ones1[:], in_=ONES[:])

        # ---- persistent activation buffers (zero-padded interiors; the
        # borders are zeroed ONCE here via engine memsets on otherwise-idle
        # engines — interior writes never touch them) ----
        act1 = [[ap_pool.tile([128, G * 676], DT_MM, tag=f"act1_{pp}_{h}",
                              name=f"act1_{pp}_{h}") for h in range(2)]
                for pp in range(2)]
        act2 = ap_pool.tile([128, BPC * 196], DT_MM)
        out3 = ap_pool.tile([64, BPC * 36], DT_MM)
        if USE_MEMSET:
            for pp in range(2):
                for h in range(2):
                    nc.vector.memset(act1[pp][h][:], 0.0)
            nc.gpsimd.memset(act2[:], 0.0)
        else:
            for pp in range(2):
                for h in range(2):
                    nc.sync.dma_start(
                        out=act1[pp][h][:],
                        in_=Z[:, :G * 676].to_broadcast((128, G * 676)))
            nc.gpsimd.dma_start(out=act2[:],
                                in_=Z[:, :BPC * 196].to_broadcast((128, BPC * 196)))

        with contextlib.ExitStack() as cctx:
            ps1 = cctx.enter_context(tc.tile_pool(name="ps1", bufs=2, space="PSUM"))
            ps2 = cctx.enter_context(tc.tile_pool(name="ps2", bufs=2, space="PSUM"))

            # conv1 chunking: pool-aligned column chunks of one image-half
            # (20 rows, 20 rows, 8 rows of the 48x48 raw conv output)
            C1_CHUNKS = [(0, 960, 20), (960, 960, 20), (1920, 384, 8)]

            def conv1_chunk(g, ci, imt, h, c):
                """One conv1 chunk: full-128-mode matmuls (K=65, taps in rows
                0-8, zeros above), then relu-first eviction + 2x2 pool."""
                lo, sz, rows = C1_CHUNKS[c]
                a1v = act1[g % 2][h][:].rearrange(
                    "p (b y x) -> p b y x", y=26, x=26)
                ps = ps1.tile([128, 1024], F32, tag="ps1", name="ps1")
                for j in range(0, sz, 512):
                    w = min(512, sz - j)
                    nc.tensor.matmul(
                        out=ps[:, j:j + w],
                        lhsT=w1t[0:73, 128 * h:128 * (h + 1)],
                        rhs=imt[0:73, 2304 * ci + lo + j:2304 * ci + lo + j + w],
                        start=True, stop=True)
                # relu+bias from PSUM -> bf16 (rows x 48)
                blk = e1p.tile([128, 960], DT_MM, tag="blk", name="blk")
                nc.scalar.activation(
                    out=blk[:, 0:sz], in_=ps[:, 0:sz],
                    func=RELU, bias=b1t[:, h:h + 1])
                bv = blk[:, 0:sz].rearrange("p (y x t) -> p y x t", x=24, t=2)
                sbx = e1p.tile([128, 480], DT_MM, tag="sbx", name="sbx")
                sxv = sbx[:, 0:sz // 2].rearrange("p (y x) -> p y x", x=24)
                nc.vector.tensor_max(sxv, bv[:, :, :, 0], bv[:, :, :, 1])
                sxp = sbx[:, 0:sz // 2].rearrange("p (y t x) -> p y t x",
                                                  t=2, x=24)
                # chunk c's pooled rows land at act1 interior rows
                # 1 + 10c .. + rows/2 (cols 1:25)
                dst = a1v[:, ci, 1 + 10 * c:1 + 10 * c + rows // 2, 1:25]
                nc.vector.tensor_max(
                    dst, sxp[:, :, 0, :], sxp[:, :, 1, :])

            def conv2_bursts(g, bb):
                """conv2 for image bb of group g as 6 sub-burst closures of
                6 matmuls each (2 psum groups of 18); the last also evicts."""
                a1vs = [act1[g % 2][h][:].rearrange(
                    "p (b y x) -> p b y x", y=26, x=26) for h in range(2)]
                a2v = act2[:].rearrange("p (b y x) -> p b y x", y=14, x=14)
                ps = ps2.tile([128, 1024], F32, tag="ps2", name="ps2")
                psr = ps[:].rearrange("p (rr c) -> p rr c", c=512)
                mms = []
                for rr in range(2):
                    for t in range(9):
                        dy, dx = divmod(t, 3)
                        for h in range(2):
                            mms.append((rr, t, h, dy, dx))

                def emit_mm(idx):
                    rr, t, h, dy, dx = mms[idx]
                    n = idx % 18
                    nc.tensor.matmul(
                        out=psr[:, rr, 0:288],
                        lhsT=w2t[:, (t * 2 + h) * 128:(t * 2 + h + 1) * 128],
                        rhs=a1vs[h][:, bb, 12 * rr + dy:12 * rr + dy + 12,
                                    dx:dx + 24],
                        start=(n == 0), stop=(n == 17))

                def evict():
                    sb2 = e2p.tile([128, 576], DT_MM, tag="sb2", name="sb2")
                    nc.scalar.activation(
                        out=sb2[:], in_=psr[:, :, 0:288],
                        func=RELU, bias=b2t[:])
                    s2v = sb2[:].rearrange("p (y x t) -> p y x t", x=12, t=2)
                    xm = e2p.tile([128, 288], DT_MM, tag="xm2", name="xm2")
                    xmv = xm[:].rearrange("p (y x) -> p y x", x=12)
                    nc.vector.tensor_max(xmv, s2v[:, :, :, 0], s2v[:, :, :, 1])
                    xmp = xm[:].rearrange("p (y t x) -> p y t x", t=2, x=12)
                    nc.vector.tensor_max(
                        a2v[:, G * g + bb, 1:13, 1:13],
                        xmp[:, :, 0, :], xmp[:, :, 1, :])

                def burst(j):
                    def run():
                        for idx in range(6 * j, 6 * j + 6):
                            emit_mm(idx)
                        if j == 5:
                            evict()
                    return run
                return [burst(j) for j in range(6)]

            # ---- group loop: conv1(g) chunks interleaved with conv2(g-1)
            # sub-bursts (the conv2 matmuls cover conv1's psum eviction
            # latency between chunks) ----
            prev_g = None
            for g in range(NG):
                if g > 0:
                    xpv = xpads[g % 2][:].rearrange("p (y x) -> p y x", x=50)
                    nc.sync.dma_start(
                        out=xpv[0:G, 1:49, 1:49],
                        in_=x[G * g:G * (g + 1), :].rearrange("b (y x) -> b y x", x=48))
                    imt = imts[g % 2]
                    issue_taps(imt, xpads[g % 2][:], g)
                else:
                    imt = imts[0]
                for ci in range(G):
                    bursts = (conv2_bursts(prev_g, ci)
                              if prev_g is not None else [None] * 6)
                    for j, (h, c) in enumerate(
                            [(h, c) for h in range(2) for c in range(3)]):
                        if bursts[j] is not None:
                            bursts[j]()
                        conv1_chunk(g, ci, imt, h, c)
                prev_g = g
            for ci in range(G):
                for b in conv2_bursts(prev_g, ci):
                    b()

        # ---- conv3 (act2 complete): 2-way column tiling ----
        with contextlib.ExitStack() as cctx:
            ps3 = cctx.enter_context(tc.tile_pool(name="ps3", bufs=3, space="PSUM"))
            psq = cctx.enter_context(tc.tile_pool(name="psq", bufs=1, space="PSUM"))
            pssc = cctx.enter_context(tc.tile_pool(name="pssc", bufs=1, space="PSUM"))
            psT = cctx.enter_context(tc.tile_pool(name="psT", bufs=1, space="PSUM"))

            a2v = act2[:].rearrange("p (b y x) -> p b y x", y=14, x=14)
            o3v = out3[:].rearrange("p (b hw) -> p b hw", hw=36)
            psq_t = psq.tile([64, 512], F32)
            pssc_t = pssc.tile([36, 512], F32)

            def attn_half(hh):
                # q accumulation + per-image scores for images 32hh..32hh+32;
                # emitted right after the conv3 rounds that produce them, so
                # these matmuls overlap conv3's remaining rounds/evictions.
                for hw in range(36):
                    nc.tensor.matmul(
                        out=psq_t[:, 32 * hh:32 * hh + 32],
                        lhsT=wet[:, 64 * hw:64 * (hw + 1)],
                        rhs=out3[:, 36 * 32 * hh + hw:36 * 32 * (hh + 1):36],
                        start=(hw == 0), stop=(hw == 35))
                q_sbh = ap_pool.tile([64, 32], DT_MM, tag=f"qsb{hh}",
                                     name=f"qsb{hh}")
                nc.vector.tensor_scalar_add(
                    q_sbh[:], psq_t[:, 32 * hh:32 * hh + 32], bet[:])
                for b in range(32 * hh, 32 * hh + 32):
                    nc.tensor.matmul(
                        out=pssc_t[:, b:b + 1],
                        lhsT=out3[:, 36 * b:36 * (b + 1)],
                        rhs=q_sbh[:, b - 32 * hh:b - 32 * hh + 1],
                        start=True, stop=True)

            if C3_TILED:
                for t in range(16):  # rounds of 4 images (2 col-tiled pairs)
                    ps = ps3.tile([128, 512], F32, tag="ps3", name="ps3")
                    for j in range(2):
                        for k in range(9):
                            dy, dx = divmod(k, 3)
                            nc.tensor.matmul(
                                out=ps[64 * j:64 * (j + 1), 0:288],
                                lhsT=w3t[:, 64 * k:64 * (k + 1)],
                                rhs=a2v[:, 4 * t + 2 * j:4 * t + 2 * j + 2,
                                        dy:dy + 12, dx:dx + 12],
                                start=(k == 0), stop=(k == 8),
                                tile_position=(0, 64 * j))
                    sb3 = e3p.tile([128, 288], DT_MM, tag="sb3", name="sb3")
                    nc.scalar.activation(out=sb3[:], in_=ps[:, 0:288], func=RELU,
                                         bias=b3t[:])
                    s3v = sb3[:].rearrange("p (b y x t) -> p b y x t", b=2, x=6, t=2)
                    xm = e3p.tile([128, 144], DT_MM, tag="xm3", name="xm3")
                    xmv = xm[:].rearrange("p (b y x) -> p b y x", b=2, x=6)
                    nc.vector.tensor_max(xmv, s3v[:, :, :, :, 0], s3v[:, :, :, :, 1])
                    xmp = xm[:].rearrange("p (b y t x) -> p b y t x", b=2, t=2, x=6)
                    # top pair -> out3 directly; bottom pair pools into a
                    # staging tile on partitions 64-127, then a tiny SBUF->
                    # SBUF DMA moves it down (DVE lanes are fixed).
                    dst = o3v[:, 4 * t:4 * t + 2, :].rearrange(
                        "p b (y x) -> p b y x", x=6)
                    nc.vector.tensor_max(
                        dst, xmp[0:64, :, :, 0, :], xmp[0:64, :, :, 1, :])
                    stg = e3p.tile([128, 72], DT_MM, tag="stg3", name="stg3")
                    sgv = stg[:].rearrange("p (b y x) -> p b y x", b=2, x=6)
                    nc.vector.tensor_max(
                        sgv[64:128, :, :, :],
                        xmp[64:128, :, :, 0, :], xmp[64:128, :, :, 1, :])
                    nc.gpsimd.dma_start(
                        out=out3[:, 36 * (4 * t + 2):36 * (4 * t + 4)],
                        in_=stg[64:128, :])
                    if t == 7:
                        attn_half(0)
                    elif t == 15:
                        attn_half(1)
            else:
                for t in range(32):  # one image pair per round
                    ps = ps3.tile([64, 512], F32, tag="ps3", name="ps3")
                    for k in range(9):
                        dy, dx = divmod(k, 3)
                        nc.tensor.matmul(
                            out=ps[:, 0:288],
                            lhsT=w3t[:, 64 * k:64 * (k + 1)],
                            rhs=a2v[:, 2 * t:2 * t + 2, dy:dy + 12, dx:dx + 12],
                            start=(k == 0), stop=(k == 8))
                    sb3 = e3p.tile([64, 288], DT_MM, tag="sb3", name="sb3")
                    nc.scalar.activation(out=sb3[:], in_=ps[:, 0:288], func=RELU,
                                         bias=b3t[0:64, :])
                    s3v = sb3[:].rearrange("p (b y x t) -> p b y x t", b=2, x=6, t=2)
                    xm = e3p.tile([64, 144], DT_MM, tag="xm3", name="xm3")
                    xmv = xm[:].rearrange("p (b y x) -> p b y x", b=2, x=6)
                    nc.vector.tensor_max(xmv, s3v[:, :, :, :, 0], s3v[:, :, :, :, 1])
                    xmp = xm[:].rearrange("p (b y t x) -> p b y t x", b=2, t=2, x=6)
                    dst = o3v[:, 2 * t:2 * t + 2, :].rearrange(
                        "p b (y x) -> p b y x", x=6)
                    nc.vector.tensor_max(
                        dst, xmp[:, :, :, 0, :], xmp[:, :, :, 1, :])

            if not C3_TILED:
                attn_half(0)
                attn_half(1)
            sc_sb = ap_pool.tile([36, 64], F32)
            nc.vector.tensor_copy(sc_sb[:], pssc_t[:, 0:64])
            psT_t = psT.tile([64, 512], F32)
            nc.tensor.transpose(psT_t[:, 0:36], sc_sb[:], ident[:])

            # ---- softmax over hw (free dim) ----
            mx = ap_pool.tile([64, 1], F32)
            nc.vector.tensor_reduce(out=mx[:], in_=psT_t[:, 0:36],
                                    op=mybir.AluOpType.max,
                                    axis=mybir.AxisListType.X)
            nmx = ap_pool.tile([64, 1], F32)
            nc.vector.tensor_scalar_mul(nmx[:], mx[:], -1.0)
            e_t = ap_pool.tile([64, 36], F32)
            nc.scalar.activation(out=e_t[:], in_=psT_t[:, 0:36], func=EXP, bias=nmx[:])
            z = ap_pool.tile([64, 1], F32)
            nc.vector.tensor_reduce(out=z[:], in_=e_t[:],
                                    op=mybir.AluOpType.add,
                                    axis=mybir.AxisListType.X)
            rz = ap_pool.tile([64, 1], F32)
            nc.vector.reciprocal(rz[:], z[:])
            attn = ap_pool.tile([64, 36], DT_MM)
            nc.vector.tensor_scalar_mul(attn[:], e_t[:], rz[:])
            if debug:
                for pp in range(2):
                    for h in range(2):
                        nc.gpsimd.dma_start(out=dbg[f"dbg_act1_p{pp}h{h}"][:],
                                            in_=act1[pp][h][:])
                nc.gpsimd.dma_start(out=dbg["dbg_act2"][:], in_=act2[:])
                nc.gpsimd.dma_start(out=dbg["dbg_out3"][:], in_=out3[:])
                nc.gpsimd.dma_start(out=dbg["dbg_attn"][:], in_=attn[:])
                nc.sync.dma_start(out=dbg["dbg_sc"][:], in_=sc_sb[:])

        # ---- g_mod + fc3 ----
        with contextlib.ExitStack() as cctx:
            psab = cctx.enter_context(tc.tile_pool(name="psab", bufs=1, space="PSUM"))
            psf = cctx.enter_context(tc.tile_pool(name="psf", bufs=1, space="PSUM"))

            attn_flat = ap_pool.tile([1, 2304], DT_MM)
            nc.sync.dma_start(out=attn_flat[:], in_=attn[:])
            psab_t = psab.tile([64, 2304], F32)
            for c in range(5):
                lo = 512 * c
                hi = min(lo + 512, 2304)
                nc.tensor.matmul(out=psab_t[:, lo:hi], lhsT=ones1[:],
                                 rhs=attn_flat[:, lo:hi], start=True, stop=True)
            # in-place: out3 is not needed after this product
            nc.vector.tensor_mul(out3[:], out3[:], psab_t[:])
            gT = ap_pool.tile([64, 64], DT_MM)
            with nc.allow_low_precision(reason="bf16 attn-weighted sum"):
                nc.vector.tensor_reduce(
                    out=gT[:], in_=out3[:].rearrange("p (b hw) -> p b hw", hw=36),
                    op=mybir.AluOpType.add, axis=mybir.AxisListType.X)

            if debug:
                nc.gpsimd.dma_start(out=dbg["dbg_gT"][:], in_=gT[:])
            psf_t = psf.tile([64, 7], F32)
            nc.tensor.matmul(out=psf_t[:], lhsT=gT[:],
                             rhs=w3f[:], start=True, stop=True)
            out_sb = ap_pool.tile([64, 7], F32)
            nc.vector.tensor_add(out_sb[:], psf_t[:], fc3b_t[:])
            nc.sync.dma_start(out=out[:], in_=out_sb[:])

    _split_excess_waits(nc)
    return nc


def kernel(**inputs):
    from concourse.bass_utils import run_bass_kernel_spmd

    w = _prep_weights({k: np.asarray(v, np.float32) for k, v in inputs.items()
                       if k != 'x'})
    npdt = mybir.dt.np(DT_MM)
    for k in ('W1T', 'W2T', 'W3T', 'WeT2', 'W3fT', 'Z', 'ONES'):
        w[k] = w[k].astype(npdt)
    xs = np.asarray(inputs['x'], np.float32).reshape(B_TOTAL, 2304).astype(npdt)

    nc = build_program()
    in_maps = []
    for c in range(N_CORES):
        m = {'x': np.ascontiguousarray(xs[BPC * c:BPC * (c + 1)])}
        m.update({k: v for k, v in w.items()})
        in_maps.append(m)
    res = run_bass_kernel_spmd(nc, in_maps, list(range(N_CORES)))
    outs = [res.results[c]['out'] for c in range(N_CORES)]
    return np.concatenate(outs, axis=0).astype(np.float32)


if __name__ == '__main__':
    rng = np.random.default_rng(0)
    fake = {
        'x': rng.standard_normal((512, 1, 48, 48), dtype=np.float32),
        'conv1_w': rng.standard_normal((256, 1, 3, 3), dtype=np.float32) * 0.05,
        'conv1_b': np.zeros(256, np.float32),
        'bn1_g': np.ones(256, np.float32), 'bn1_b': np.zeros(256, np.float32),
        'bn1_m': np.zeros(256, np.float32), 'bn1_v': np.ones(256, np.float32),
        'conv2_w': rng.standard_normal((128, 256, 3, 3), dtype=np.float32) * 0.05,
        'conv2_b': np.zeros(128, np.float32),
        'bn2_g': np.ones(128, np.float32), 'bn2_b': np.zeros(128, np.float32),
        'bn2_m': np.zeros(128, np.float32), 'bn2_v': np.ones(128, np.float32),
        'conv3_w': rng.standard_normal((64, 128, 3, 3), dtype=np.float32) * 0.05,
        'conv3_b': np.zeros(64, np.float32),
        'bn3_g': np.ones(64, np.float32), 'bn3_b': np.zeros(64, np.float32),
        'bn3_m': np.zeros(64, np.float32), 'bn3_v': np.ones(64, np.float32),
        'fc1_w': rng.standard_normal((512, 2304), dtype=np.float32) * 0.05,
        'fc1_b': np.zeros(512, np.float32),
        'fc2_w': rng.standard_normal((256, 512), dtype=np.float32) * 0.05,
        'fc2_b': np.zeros(256, np.float32),
        'att_w': rng.standard_normal((64, 256), dtype=np.float32) * 0.05,
        'att_b': np.zeros(64, np.float32),
        'fc3_w': rng.standard_normal((7, 64), dtype=np.float32) * 0.05,
        'fc3_b': np.zeros(7, np.float32),
    }
    print(kernel(**fake).shape)
